# revision 1
# baseline (speedup 1.0000x reference)
"""Trainium2 Bass kernel for nn_Network_67388036874689.

Data-parallel over batch: B=256 sharded as 32 samples on each of 8 cores;
all parameters replicated.

Structure exploited (validated against the reference on host):
  - fog_of_war's greedy scan returns arange(B) -> the permutation is identity.
  - conv2d(3x3, pad=1) on [C, H, 1] spatial input only sees kernel column 1
    -> 1D conv over H with 3 taps.
  - Embedding lookup (V=14) followed by pair-maxpool = lookup into a 196-entry
    pairwise-max table, implemented as one-hot matmuls on the PE.
  - The manipulator conv input is constant over H -> collapses to 3 matmuls
    (interior / h=0 / h=127 tap-sum variants).

Precision: critical path to the token discretization (enemy branch + manip)
in fp32 / float32r; post-token friend branch in bf16.
"""

import numpy as np
import ml_dtypes
from contextlib import ExitStack

import concourse.bass as bass
import concourse.bacc as bacc
import concourse.mybir as mybir
import concourse.tile as tile
from concourse.masks import make_identity
from concourse.bass_utils import run_bass_kernel_spmd

F32 = mybir.dt.float32
F32R = mybir.dt.float32r
BF16 = mybir.dt.bfloat16
I32 = mybir.dt.int32
AF = mybir.ActivationFunctionType
ALU = mybir.AluOpType
AX = mybir.AxisListType

NCORES = 8
B = 256
BC = B // NCORES        # 32 samples per core
L = 256                 # sequence length
V = 14                  # vocab
EMB = 512               # embedding dim
H = L // 2              # 128 pooled positions
NPAIR = V * V           # 196
PAIR0 = 112             # pair-table chunk split: 112 (t0 0..7) + 84 (t0 8..13)
PAIR1 = NPAIR - PAIR0   # 84
DEBUG_TAPS = False      # add intermediate DRAM outputs for debugging
SLAB = 8                # samples per embed/pool slab group
NGRP = BC // SLAB       # 4 groups
SLABW = SLAB * (H + 1) + 1   # padded slab width (stride 129 per sample)


def _dram_inputs(nc):
    t = {}

    def inp(name, shape, dt):
        t[name] = nc.dram_tensor(name, list(shape), dt, kind="ExternalInput").ap()

    inp("x", (BC, L), I32)
    inp("eemb", (V, EMB), F32R)
    inp("ecw", (256, 512 * 3), F32)       # enemy conv center col, [o, i*3+dh]
    inp("ecb", (256,), F32)
    inp("elw", (32768, 128), F32)
    inp("elb", (128,), F32)
    inp("mcw", (64, 128 * 3), F32)        # manip conv center col
    inp("mcb", (64,), F32)
    inp("mlw", (8192, 256), F32R)
    inp("mlb", (256,), F32)
    inp("femb", (V, EMB), BF16)
    inp("fcw", (256, 512 * 3), BF16)
    inp("fcb", (256,), F32)
    inp("flw", (32768, 128), BF16)
    inp("flb", (128,), F32)
    inp("f2w", (128, 14), F32)
    inp("f2b", (14,), F32)
    t["out"] = nc.dram_tensor("out", [BC, 14], F32, kind="ExternalOutput").ap()
    return t


def _tap(nc, io, name, ap):
    if not DEBUG_TAPS:
        return
    shape = list(ap.shape)
    t = nc.dram_tensor("tap_" + name, shape, ap.dtype, kind="ExternalOutput").ap()
    io["tap_" + name] = t
    nc.gpsimd.dma_start(t, ap)


def build_kernel(nc, tc, ctx):
    io = _dram_inputs(nc)
    consts = ctx.enter_context(tc.tile_pool(name="consts", bufs=1))
    work = ctx.enter_context(tc.tile_pool(name="work", bufs=1))
    wpool = ctx.enter_context(tc.tile_pool(name="wstream", bufs=8))
    prep = ctx.enter_context(tc.tile_pool(name="prep", bufs=2))
    psum_emb = ctx.enter_context(tc.tile_pool(name="psum_emb", bufs=4, space="PSUM"))
    psum_conv = ctx.enter_context(tc.tile_pool(name="psum_conv", bufs=2, space="PSUM"))
    psum_lin = ctx.enter_context(tc.tile_pool(name="psum_lin", bufs=1, space="PSUM"))
    psum_sm = ctx.enter_context(tc.tile_pool(name="psum_sm", bufs=1, space="PSUM"))

    def ctile(shape, dt, tag):
        return consts.tile(shape, dt, tag=tag, name=tag)

    def wtile(shape, dt, tag):
        return work.tile(shape, dt, tag=tag, name=tag)

    # ---------------- constants ----------------
    identF = ctile([128, 128], F32, "identF")
    make_identity(nc, identF)
    identB = ctile([128, 128], BF16, "identB")
    make_identity(nc, identB)
    iota_i = ctile([128, 1], I32, "iota_i")
    nc.gpsimd.iota(iota_i[:, :], pattern=[[0, 1]], base=0, channel_multiplier=1)
    iota_col = ctile([128, 1], F32, "iota_col")
    nc.vector.tensor_copy(iota_col[:, :], iota_i[:, :])
    ones_col = ctile([128, 1], F32, "ones_col")
    nc.vector.memset(ones_col[:, :], 1.0)
    ones_row = ctile([1, 128], F32, "ones_row")
    nc.vector.memset(ones_row[:, :], 1.0)
    iota_row = ctile([1, 128], F32, "iota_row")
    nc.gpsimd.dma_start(iota_row[:, :], iota_col[:, :])
    e0_row = ctile([1, 128], F32, "e0_row")
    nc.vector.tensor_scalar(e0_row[:, :], iota_row[:, :], 0.0, None, ALU.is_equal)
    eL_row = ctile([1, 128], F32, "eL_row")
    nc.vector.tensor_scalar(eL_row[:, :], iota_row[:, :], 127.0, None, ALU.is_equal)
    ei_row = ctile([1, 128], F32, "ei_row")
    nc.vector.scalar_tensor_tensor(ei_row[:, :], e0_row[:, :], -1.0, eL_row[:, :],
                                   ALU.mult, ALU.subtract)
    nc.vector.tensor_scalar(ei_row[:, :], ei_row[:, :], 1.0, None, ALU.add)
    zpad = ctile([128, 32], F32, "zpad")
    nc.vector.memset(zpad[:, :], 0.0)
    zpadr = ctile([128, 32], F32R, "zpadr")
    nc.vector.tensor_copy(zpadr[:, :], zpad[:, :])
    zpadb = ctile([128, 32], BF16, "zpadb")
    nc.vector.tensor_copy(zpadb[:, :], zpad[:, :])

    def bias_col(dram_vec, n, tag):
        col = ctile([n, 1], F32, tag)
        nc.gpsimd.dma_start(col[:, :], dram_vec)
        return col

    def bias_bcast(dram_vec, rows, width, tag):
        out = ctile([rows, width], F32, tag)
        nc.gpsimd.dma_start(out[:, :], dram_vec[None, :].partition_broadcast(rows))
        return out

    EBc = bias_bcast(io["ecb"], 128, 256, "EB")
    FBc = bias_bcast(io["fcb"], 128, 256, "FB")
    MBc = bias_bcast(io["mlb"], BC, 256, "MB")
    F2Bc = bias_bcast(io["f2b"], BC, 14, "F2B")
    elb_col = bias_col(io["elb"], 128, "elb")
    flb_col = bias_col(io["flb"], 128, "flb")
    mcb_col = bias_col(io["mcb"], 64, "mcb")

    # pair-max tables: pm[t0, t1*512+ch] = max(emb[t0,ch], emb[t1,ch]).
    # Built as two partition-base-0 pieces (t0 0..7 / 8..13), then reshaped
    # to [pair, ch] partition chunks by SBUF->SBUF DMA (all on-chip).
    def pair_table(emb_dram, dt, tag):
        embA = ctile([8, EMB], dt, tag + "_embA")
        nc.gpsimd.dma_start(embA[:, :], emb_dram[0:8, :])
        embB = ctile([6, EMB], dt, tag + "_embB")
        nc.gpsimd.dma_start(embB[:, :], emb_dram[8:V, :])
        tps = []
        for half, esb, nt0 in (("0", embA, 8), ("1", embB, 6)):
            pm = work.tile([nt0, V * EMB], dt, tag="pm", name="pm" + half)
            for t1 in range(V):
                embt1 = prep.tile([V, EMB], dt, tag="embt1", name="embt1")
                nc.gpsimd.dma_start(embt1[:, :],
                                  emb_dram[t1, :][None, :].partition_broadcast(V))
                nc.vector.tensor_tensor(pm[:, t1 * EMB:(t1 + 1) * EMB],
                                        esb[:, :], embt1[0:nt0, :], ALU.max)
            tp = ctile([nt0 * V, EMB], dt, tag + half)
            nc.gpsimd.dma_start(tp[:, :], pm[:, :])
            tps.append(tp)
        return tps[0], tps[1]

    tpE0, tpE1 = pair_table(io["eemb"], F32R, "tpE")
    _tap(nc, io, "tpE0", tpE0[:, :])
    _tap(nc, io, "tpE1", tpE1[:, :])
    tpF0, tpF1 = pair_table(io["femb"], BF16, "tpF")

    # conv weights -> 4 tiles [128 i, dh*256 + o] per branch
    def conv_wt(cw_dram, load_dt, wt_dt, ident, tag):
        wts = [ctile([128, 3 * 256], wt_dt, f"{tag}{kc}") for kc in range(4)]
        for oc in range(2):
            wsb = work.tile([128, 1536], load_dt, tag="pm", name="wsb")
            nc.gpsimd.dma_start(wsb[:, :], cw_dram[oc * 128:(oc + 1) * 128, :])
            for kc in range(4):
                for dh in range(3):
                    tp = psum_sm.tile([128, 128], load_dt, tag="sm", name="tpsum")
                    src = wsb[:, (kc * 128 * 3 + dh):((kc + 1) * 128 * 3):3]
                    nc.tensor.transpose(tp[:, :], src, ident)
                    nc.vector.tensor_copy(
                        wts[kc][:, dh * 256 + oc * 128: dh * 256 + (oc + 1) * 128],
                        tp[:, :])
        return wts

    wtE = conv_wt(io["ecw"], F32, F32R, identF, "wtE")
    wtF = conv_wt(io["fcw"], BF16, BF16, identB, "wtF")
    for kc in range(4):
        _tap(nc, io, f"wtE{kc}", wtE[kc][:, :])
        _tap(nc, io, f"wtF{kc}", wtF[kc][:, :])

    # manip tap-sum weights, transposed to [128 i, 64 o]
    wMsb = wtile([64, 384], F32, "wMsb")
    nc.gpsimd.dma_start(wMsb[:, :], io["mcw"])
    s01 = wtile([64, 128], F32, "s01")
    nc.vector.tensor_tensor(s01[:, :], wMsb[:, 0:384:3], wMsb[:, 1:384:3], ALU.add)
    s12 = wtile([64, 128], F32, "s12")
    nc.vector.tensor_tensor(s12[:, :], wMsb[:, 1:384:3], wMsb[:, 2:384:3], ALU.add)
    sint = wtile([64, 128], F32, "sint")
    nc.vector.tensor_tensor(sint[:, :], s01[:, :], wMsb[:, 2:384:3], ALU.add)
    wsumT = {}
    for name, src in (("int", sint), ("h0", s12), ("hL", s01)):
        tp = psum_sm.tile([128, 64], F32, tag="sm", name="tpsum")
        nc.tensor.transpose(tp[:, :], src[:, :], identF[0:64, 0:64])
        wsumT[name] = ctile([128, 64], F32R, f"wsumT_{name}")
        nc.vector.tensor_copy(wsumT[name][:, :], tp[:, :])

    # ---------------- shared stage helpers ----------------
    def embed_pool_grp(idx_row, g, tp0, tp1, slab_dt, tag):
        """Group g (8 samples): one-hot embed + pair-max -> 4 padded slabs."""
        slabs = [work.tile([128, SLABW], slab_dt, tag=f"slab{kc}",
                           name=f"slab{kc}") for kc in range(4)]
        zsrc = zpadb if slab_dt == BF16 else zpadr
        npad = SLAB + 1
        for kc in range(4):
            nc.vector.tensor_copy(slabs[kc][:, 0:SLABW:H + 1], zsrc[:, 0:npad])
        npos = SLAB * H  # 1024
        oh0 = work.tile([PAIR0, npos], slab_dt, tag="oh0", name="oh0")
        oh1 = work.tile([PAIR1, npos], slab_dt, tag="oh1", name="oh1")
        for nt in range(npos // 512):
            idxpp = psum_emb.tile([PAIR0, 512], F32, tag="pp", name="idxpp")
            nc.tensor.matmul(idxpp[:, :], ones_row[:, 0:PAIR0],
                             idx_row[:, g * npos + nt * 512:
                                     g * npos + (nt + 1) * 512],
                             start=True, stop=True)
            nc.vector.tensor_scalar(oh0[:, nt * 512:(nt + 1) * 512],
                                    idxpp[:, :], iota_col[0:PAIR0, :],
                                    None, ALU.is_equal)
            nc.vector.tensor_scalar(oh1[:, nt * 512:(nt + 1) * 512],
                                    idxpp[0:PAIR1, :], float(PAIR0),
                                    iota_col[0:PAIR1, :], ALU.subtract,
                                    ALU.is_equal)
        mm0, mm1, mo0, mo1 = tp0, tp1, oh0, oh1
        _tap(nc, io, f"{tag}_g{g}oh0", oh0[:, :])
        for kc in range(4):
            for nt in range(npos // 512):
                pp = psum_emb.tile([128, 512], F32, tag="pp", name="pp")
                nc.tensor.matmul(pp[:, :], mm0[:, kc * 128:(kc + 1) * 128],
                                 mo0[:, nt * 512:(nt + 1) * 512],
                                 start=True, stop=False)
                nc.tensor.matmul(pp[:, :], mm1[:, kc * 128:(kc + 1) * 128],
                                 mo1[:, nt * 512:(nt + 1) * 512],
                                 start=False, stop=True)
                # scatter 4 samples x 128 positions into the padded slab
                s0 = nt * 4
                dst = slabs[kc][:, 1 + s0 * (H + 1): 1 + (s0 + 4) * (H + 1)] \
                    .rearrange("p (s w) -> p s w", w=H + 1)[:, :, 0:H]
                nc.vector.tensor_copy(
                    dst, pp[:, :].rearrange("p (s w) -> p s w", w=H))
        for kc in range(4):
            _tap(nc, io, f"{tag}_g{g}slab{kc}", slabs[kc][:, :])
        return slabs

    def conv_grp(slabs, g, wts, bias_bc, acts, acts_dt):
        """3-tap conv for the 8 samples of group g; write biased acts."""
        for ls in range(SLAB):
            s = g * SLAB + ls
            cp = psum_conv.tile([128, 256], F32, tag="cp", name="cp")
            first = True
            for kc in range(4):
                for dh in range(3):
                    lhsT = slabs[kc][:, ls * (H + 1) + dh: ls * (H + 1) + dh + 128]
                    rhs = wts[kc][:, dh * 256:(dh + 1) * 256]
                    nc.tensor.matmul(cp[:, :], lhsT, rhs,
                                     start=first, stop=(kc == 3 and dh == 2))
                    first = False
            nc.vector.tensor_tensor(acts[:, s * 256:(s + 1) * 256],
                                    cp[:, :], bias_bc[:, :], ALU.add)
            if DEBUG_TAPS and s == 28 and acts.dtype != BF16:
                dbg = work.tile([128, 256], F32, tag="dbgcp", name="dbgcp")
                nc.vector.tensor_copy(dbg[:, :], cp[:, :])
                _tap(nc, io, "cp28", dbg[:, :])

    def big_linear(acts, w_dram, wdt, tag):
        """psum[j(128), b(32)] = sum_c W_c^T @ acts[:, (b, o=c)]."""
        lp = psum_lin.tile([128, BC], F32, tag="lp", name=f"{tag}_lp")
        for c in range(256):
            wsb = wpool.tile([128, 128], wdt, tag="w", name="w")
            nc.gpsimd.dma_start(wsb[:, :], w_dram[c * 128:(c + 1) * 128, :])
            rhs = acts[:, c:c + (BC - 1) * 256 + 1:256]
            nc.tensor.matmul(lp[:, :], wsb[:, :], rhs,
                             start=(c == 0), stop=(c == 255))
        return lp

    # ---------------- enemy branch ----------------
    xsb = wtile([BC, L], I32, "xsb")
    nc.gpsimd.dma_start(xsb[:, :], io["x"])
    xf = wtile([BC, L], F32, "xf")
    nc.vector.tensor_copy(xf[:, :], xsb[:, :])
    idxE = wtile([BC, H], F32, "idxE")
    nc.vector.scalar_tensor_tensor(idxE[:, :], xf[:, 0:L:2], float(V),
                                   xf[:, 1:L:2], ALU.mult, ALU.add)
    idxrowE = wtile([1, BC * H], F32, "idxrow")
    nc.gpsimd.dma_start(idxrowE[:, :], idxE[:, :])
    _tap(nc, io, "idxrowE", idxrowE[:, :])
    _tap(nc, io, "idxE", idxE[:, :])

    actsE = wtile([128, BC * 256], F32, "actsE")
    for g in range(NGRP):
        slabs = embed_pool_grp(idxrowE, g, tpE0, tpE1, F32R, "E")
        conv_grp(slabs, g, wtE, EBc, actsE, F32)

    _tap(nc, io, "actsE", actsE[:, :])
    lpE = big_linear(actsE, io["elw"], F32, "E")
    # softmax over j (partition dim): exp, sum via matmul, normalize
    Ex = wtile([128, BC], F32, "Ex")
    nc.scalar.activation(Ex[:, :], lpE[:, :], AF.Exp, bias=elb_col[:, :])
    s1 = psum_sm.tile([BC, 1], F32, tag="sm", name="s1")
    nc.tensor.matmul(s1[:, :], Ex[:, :], ones_col[:, :], start=True, stop=True)
    r32 = wtile([BC, 1], F32, "r32")
    nc.vector.reciprocal(r32[:, :], s1[:, :])
    rrow = wtile([1, BC], F32, "rrow")
    nc.gpsimd.dma_start(rrow[:, :], r32[:, :])
    rbp = psum_sm.tile([128, BC], F32, tag="sm", name="rbp")
    nc.tensor.matmul(rbp[:, :], ones_row[:, :], rrow[:, :], start=True, stop=True)
    _tap(nc, io, "Ex", Ex[:, :])
    vT = wtile([128, BC], F32R, "vT")   # enemy_out^T [i, b]
    nc.vector.tensor_tensor(vT[:, :], Ex[:, :], rbp[:, :], ALU.mult)

    # ---------------- manipulator ----------------
    rowsb = {}
    for name in ("int", "h0", "hL"):
        cx = psum_sm.tile([64, BC], F32, tag="sm", name="cx")
        nc.tensor.matmul(cx[:, :], wsumT[name][:, :],
                         vT[:, :], start=True, stop=True)
        cxs = work.tile([64, BC], F32, tag=f"cxs_{name}", name=f"cxs_{name}")
        nc.scalar.activation(cxs[:, :], cx[:, :], AF.Relu, bias=mcb_col[:, :])
        rowsb[name] = work.tile([1, 64 * BC], F32, tag="pm" if name == "int" else f"row_{name}",
                                name=f"row_{name}")
        nc.gpsimd.dma_start(rowsb[name][:, :], cxs[:, :])
    # assemble [128 h, (o, b)] manip acts: rows 1..126 = interior variant,
    # row 0 = h0 variant, row 127 = hL variant, via K=1 mask matmuls
    acts_m = wtile([128, 64 * BC], F32R, "acts_m")
    for nt in range(64 * BC // 512):
        amp = psum_emb.tile([128, 512], F32, tag="pp", name="amp")
        sl = slice(nt * 512, (nt + 1) * 512)
        nc.tensor.matmul(amp[:, :], ei_row[:, :], rowsb["int"][:, sl],
                         start=True, stop=False)
        nc.tensor.matmul(amp[:, :], e0_row[:, :], rowsb["h0"][:, sl],
                         start=False, stop=False)
        nc.tensor.matmul(amp[:, :], eL_row[:, :], rowsb["hL"][:, sl],
                         start=False, stop=True)
        nc.vector.tensor_copy(acts_m[:, sl], amp[:, :])

    mp = psum_lin.tile([BC, 256], F32, tag="lp", name="mp")
    for c in range(64):
        wsb = wpool.tile([128, 256], F32R, tag="w", name="w")
        nc.gpsimd.dma_start(wsb[:, :], io["mlw"][c * 128:(c + 1) * 128, :])
        nc.tensor.matmul(mp[:, :], acts_m[:, c * BC:(c + 1) * BC], wsb[:, :],
                         start=(c == 0), stop=(c == 63))
    m_sb = wtile([BC, 256], F32, "m_sb")
    nc.vector.tensor_tensor(m_sb[:, :], mp[:, :], MBc[0:BC, :], ALU.add)
    _tap(nc, io, "m", m_sb[:, :])

    # tokens = floor(|m|*100) mod 14; pair idx = 14*even + odd
    # floor via the 2^23 magic-number trick (t in [0, ~50) << 2^23):
    #   round_nearest(t - 0.5 + 2^23) - 2^23 == floor(t) for non-integer t
    # mod 14 via repeated conditional subtract (covers t < 42)
    tt = wtile([BC, 256], F32, "tt")
    nc.scalar.activation(tt[:, :], m_sb[:, :], AF.Abs, scale=100.0)
    fu = wtile([BC, 256], F32, "fu")
    nc.vector.tensor_scalar(fu[:, :], tt[:, :], 8388607.5, None, ALU.add)
    fr = wtile([BC, 256], F32, "fr")
    nc.vector.tensor_scalar(fr[:, :], fu[:, :], 8388608.0, None, ALU.subtract)
    ti = wtile([BC, 256], F32, "ti")
    nc.vector.tensor_scalar(ti[:, :], fr[:, :], float(V), None, ALU.is_ge)
    t1 = wtile([BC, 256], F32, "t1")
    nc.vector.scalar_tensor_tensor(t1[:, :], ti[:, :], -float(V), fr[:, :],
                                   ALU.mult, ALU.add)
    t2 = wtile([BC, 256], F32, "t2")
    nc.vector.tensor_scalar(t2[:, :], t1[:, :], float(V), None, ALU.is_ge)
    tok = wtile([BC, 256], F32, "tok")
    nc.vector.scalar_tensor_tensor(tok[:, :], t2[:, :], -float(V), t1[:, :],
                                   ALU.mult, ALU.add)
    _tap(nc, io, "tok", tok[:, :])
    idxF = wtile([BC, H], F32, "idxF")
    nc.vector.scalar_tensor_tensor(idxF[:, :], tok[:, 0:256:2], float(V),
                                   tok[:, 1:256:2], ALU.mult, ALU.add)
    idxrowF = wtile([1, BC * H], F32, "idxrow")
    nc.gpsimd.dma_start(idxrowF[:, :], idxF[:, :])

    # ---------------- friend branch (bf16) ----------------
    actsF = wtile([128, BC * 256], BF16, "actsF")
    for g in range(NGRP):
        slabs = embed_pool_grp(idxrowF, g, tpF0, tpF1, BF16, "F")
        conv_grp(slabs, g, wtF, FBc, actsF, BF16)

    _tap(nc, io, "actsF", actsF[:, :])
    _tap(nc, io, "vT", vT[:, :])
    lpF = big_linear(actsF, io["flw"], BF16, "F")
    fsb = wtile([128, BC], F32, "fsb")
    nc.vector.tensor_scalar(fsb[:, :], lpF[:, :], flb_col[:, :], None, ALU.add)

    w2sb = wtile([128, 14], F32, "w2sb")
    nc.gpsimd.dma_start(w2sb[:, :], io["f2w"])
    f2 = psum_sm.tile([BC, 14], F32, tag="sm", name="f2")
    nc.tensor.matmul(f2[:, :], fsb[:, :], w2sb[:, :], start=True, stop=True)
    logits = wtile([BC, 14], F32, "logits")
    nc.vector.tensor_tensor(logits[:, :], f2[:, :], F2Bc[0:BC, :], ALU.add)
    nmx = wtile([BC, 1], F32, "nmx")
    nc.vector.reduce_max(nmx[:, :], logits[:, :], AX.X, negate=True)
    ex = wtile([BC, 14], F32, "ex")
    nc.scalar.activation(ex[:, :], logits[:, :], AF.Exp, bias=nmx[:, :])
    sm = wtile([BC, 1], F32, "sm")
    nc.vector.reduce_sum(sm[:, :], ex[:, :], AX.X)
    rs = wtile([BC, 1], F32, "rs")
    nc.vector.reciprocal(rs[:, :], sm[:, :])
    outt = wtile([BC, 14], F32, "outt")
    nc.vector.tensor_scalar(outt[:, :], ex[:, :], rs[:, :], None, ALU.mult)
    nc.gpsimd.dma_start(io["out"], outt[:, :])


_CACHE = {}


def _get_nc():
    if "nc" not in _CACHE:
        nc = bacc.Bacc("TRN2", target_bir_lowering=False, debug=False,
                       num_devices=NCORES)
        with tile.TileContext(nc) as tc:
            with ExitStack() as ctx:
                build_kernel(nc, tc, ctx)
        nc.compile()
        _CACHE["nc"] = nc
    return _CACHE["nc"]


def prep_inputs(inputs):
    """Host-side shard/layout prep. Returns list of 8 in_maps."""
    f32 = np.float32
    bf16 = ml_dtypes.bfloat16
    common = {
        "eemb": np.ascontiguousarray(inputs["enemy_emb"], f32),
        "ecw": np.ascontiguousarray(
            np.asarray(inputs["enemy_conv_w"])[:, :, :, 1], f32).reshape(256, -1),
        "ecb": np.ascontiguousarray(inputs["enemy_conv_b"], f32),
        "elw": np.ascontiguousarray(inputs["enemy_lin_w"], f32),
        "elb": np.ascontiguousarray(inputs["enemy_lin_b"], f32),
        "mcw": np.ascontiguousarray(
            np.asarray(inputs["manip_conv_w"])[:, :, :, 1], f32).reshape(64, -1),
        "mcb": np.ascontiguousarray(inputs["manip_conv_b"], f32),
        "mlw": np.ascontiguousarray(inputs["manip_lin_w"], f32),
        "mlb": np.ascontiguousarray(inputs["manip_lin_b"], f32),
        "femb": np.asarray(inputs["friend_emb"]).astype(bf16),
        "fcw": np.ascontiguousarray(
            np.asarray(inputs["friend_conv_w"])[:, :, :, 1]).reshape(256, -1)
            .astype(bf16),
        "fcb": np.ascontiguousarray(inputs["friend_conv_b"], f32),
        "flw": np.asarray(inputs["friend_lin1_w"]).astype(bf16),
        "flb": np.ascontiguousarray(inputs["friend_lin1_b"], f32),
        "f2w": np.ascontiguousarray(inputs["friend_lin2_w"], f32),
        "f2b": np.ascontiguousarray(inputs["friend_lin2_b"], f32),
    }
    x = np.ascontiguousarray(inputs["x"], np.int32)
    return [dict(common, x=np.ascontiguousarray(x[c * BC:(c + 1) * BC]))
            for c in range(NCORES)]


def kernel(**inputs):
    nc = _get_nc()
    in_maps = prep_inputs(inputs)
    res = run_bass_kernel_spmd(nc, in_maps, core_ids=list(range(NCORES)))
    return np.concatenate([r["out"] for r in res.results], axis=0)



# revision 6
# speedup vs baseline: 1.7800x; 1.7800x over previous
"""Trainium2 Bass kernel for nn_Network_67388036874689.

Data-parallel over batch: B=256 sharded as 32 samples on each of 8 cores;
all parameters replicated.

Structure exploited (validated against the reference on host):
  - fog_of_war's greedy scan returns arange(B) -> the permutation is identity.
  - conv2d(3x3, pad=1) on [C, H, 1] spatial input only sees kernel column 1
    -> 1D conv over H with 3 taps.
  - Embedding lookup (V=14) followed by pair-maxpool = lookup into a 196-entry
    pairwise-max table, implemented as one-hot matmuls on the PE.
  - The manipulator conv input is constant over H -> collapses to 3 matmuls
    (interior / h=0 / h=127 tap-sum variants).

Performance structure:
  - Conv / tap-sum weight transposes are done host-side in prep_inputs.
  - The three big weight streams (elw 16.8MB, mlw 8.4MB, flw 8.4MB) are
    DMA'd in large batched transfers on the two HWDGE queues (sync carries
    elw, scalar carries mlw then flw) with ring buffers, so they prefetch
    underneath the conv phases instead of gating the linear phases.
  - friend_lin1_w is pair-packed host-side so every DMA descriptor is 512B.

Precision: critical path to the token discretization (enemy branch + manip)
in fp32 / float32r; post-token friend branch in bf16.
"""

import numpy as np
import ml_dtypes
from contextlib import ExitStack

import concourse.bass as bass
import concourse.bacc as bacc
import concourse.mybir as mybir
import concourse.tile as tile
from concourse.bass_utils import run_bass_kernel_spmd

F32 = mybir.dt.float32
F32R = mybir.dt.float32r
BF16 = mybir.dt.bfloat16
I32 = mybir.dt.int32
AF = mybir.ActivationFunctionType
ALU = mybir.AluOpType
AX = mybir.AxisListType

NCORES = 8
B = 256
BC = B // NCORES        # 32 samples per core
L = 256                 # sequence length
V = 14                  # vocab
EMB = 512               # embedding dim
H = L // 2              # 128 pooled positions
NPAIR = V * V           # 196
PAIR0 = 112             # pair-table chunk split: 112 (t0 0..7) + 84 (t0 8..13)
PAIR1 = NPAIR - PAIR0   # 84
SLAB = 8                # samples per embed/pool slab group
NGRP = BC // SLAB       # 4 groups
SLABW = SLAB * (H + 1) + 1   # padded slab width (stride 129 per sample)


def _dram_inputs(nc):
    t = {}

    def inp(name, shape, dt):
        t[name] = nc.dram_tensor(name, list(shape), dt, kind="ExternalInput").ap()

    inp("x", (BC, L), I32)
    inp("eemb", (V, EMB), F32R)
    inp("ecwT", (512, 3 * 256), F32R)     # [i_global, dh*256+o]
    inp("ecb", (256,), F32)
    inp("elw", (32768, 128), F32)
    inp("elb", (128,), F32)
    inp("mwT", (128, 3 * 64), F32R)       # [i, {int,h0,hL}*64+o] tap sums
    inp("mcb", (64,), F32)
    inp("mlw", (8192, 256), F32R)
    inp("mlb", (256,), F32)
    inp("femb", (V, EMB), BF16)
    inp("fcwT", (512, 3 * 256), BF16)     # [i_global, dh*256+o]
    inp("fcb", (256,), F32)
    inp("flw2", (16384, 256), BF16)       # pair-packed friend_lin1_w
    inp("flb", (128,), F32)
    inp("f2w", (128, 14), F32)
    inp("f2b", (14,), F32)
    t["out"] = nc.dram_tensor("out", [BC, 14], F32, kind="ExternalOutput").ap()
    return t


def build_kernel(nc, tc, ctx):
    io = _dram_inputs(nc)
    consts = ctx.enter_context(tc.tile_pool(name="consts", bufs=1))
    work = ctx.enter_context(tc.tile_pool(name="work", bufs=1))
    slabp = ctx.enter_context(tc.tile_pool(name="slabp", bufs=2))
    ohp = ctx.enter_context(tc.tile_pool(name="ohp", bufs=1))
    psum_emb = ctx.enter_context(tc.tile_pool(name="psum_emb", bufs=4, space="PSUM"))
    psum_conv = ctx.enter_context(tc.tile_pool(name="psum_conv", bufs=2, space="PSUM"))
    psum_lin = ctx.enter_context(tc.tile_pool(name="psum_lin", bufs=1, space="PSUM"))
    psum_sm = ctx.enter_context(tc.tile_pool(name="psum_sm", bufs=1, space="PSUM"))

    def ctile(shape, dt, tag):
        return consts.tile(shape, dt, tag=tag, name=tag)

    def wtile(shape, dt, tag):
        return work.tile(shape, dt, tag=tag, name=tag)

    # ---------------- constants ----------------
    iota_i = ctile([128, 1], I32, "iota_i")
    nc.gpsimd.iota(iota_i[:, :], pattern=[[0, 1]], base=0, channel_multiplier=1)
    iota_col = ctile([128, 1], F32, "iota_col")
    nc.vector.tensor_copy(iota_col[:, :], iota_i[:, :])
    ones_col = ctile([128, 1], F32, "ones_col")
    nc.vector.memset(ones_col[:, :], 1.0)
    ones_row = ctile([1, 128], F32, "ones_row")
    nc.vector.memset(ones_row[:, :], 1.0)
    iota_row = ctile([1, 128], F32, "iota_row")
    nc.gpsimd.dma_start(iota_row[:, :], iota_col[:, :])
    e0_row = ctile([1, 128], F32, "e0_row")
    nc.vector.tensor_scalar(e0_row[:, :], iota_row[:, :], 0.0, None, ALU.is_equal)
    eL_row = ctile([1, 128], F32, "eL_row")
    nc.vector.tensor_scalar(eL_row[:, :], iota_row[:, :], 127.0, None, ALU.is_equal)
    ei_row = ctile([1, 128], F32, "ei_row")
    nc.vector.scalar_tensor_tensor(ei_row[:, :], e0_row[:, :], -1.0, eL_row[:, :],
                                   ALU.mult, ALU.subtract)
    nc.vector.tensor_scalar(ei_row[:, :], ei_row[:, :], 1.0, None, ALU.add)
    zpad = ctile([128, 32], F32, "zpad")
    nc.vector.memset(zpad[:, :], 0.0)
    zpadr = ctile([128, 32], F32R, "zpadr")
    nc.vector.tensor_copy(zpadr[:, :], zpad[:, :])
    zpadb = ctile([128, 32], BF16, "zpadb")
    nc.vector.tensor_copy(zpadb[:, :], zpad[:, :])

    def bias_col(dram_vec, n, tag):
        col = ctile([n, 1], F32, tag)
        nc.gpsimd.dma_start(col[:, :], dram_vec)
        return col

    def bias_bcast(dram_vec, rows, width, tag):
        out = ctile([rows, width], F32, tag)
        nc.gpsimd.dma_start(out[:, :], dram_vec[None, :].partition_broadcast(rows))
        return out

    EBc = bias_bcast(io["ecb"], 128, 256, "EB")
    FBc = bias_bcast(io["fcb"], 128, 256, "FB")
    MBc = bias_bcast(io["mlb"], BC, 256, "MB")
    F2Bc = bias_bcast(io["f2b"], BC, 14, "F2B")
    elb_col = bias_col(io["elb"], 128, "elb")
    flb_col = bias_col(io["flb"], 128, "flb")
    mcb_col = bias_col(io["mcb"], 64, "mcb")

    # conv weights, already transposed host-side: 4 tiles [128 i, dh*256+o]
    wtE_all = ctile([128, 4 * 768], F32R, "wtE_all")
    nc.sync.dma_start(wtE_all[:, :].rearrange("p (kc d) -> p kc d", d=768),
                      io["ecwT"].rearrange("(kc p) d -> p kc d", p=128))
    wtE = [wtE_all[:, kc * 768:(kc + 1) * 768] for kc in range(4)]
    wtF_all = ctile([128, 4 * 768], BF16, "wtF_all")
    nc.scalar.dma_start(wtF_all[:, :].rearrange("p (kc d) -> p kc d", d=768),
                        io["fcwT"].rearrange("(kc p) d -> p kc d", p=128))
    wtF = [wtF_all[:, kc * 768:(kc + 1) * 768] for kc in range(4)]

    # manip tap-sum weights, host-transposed: [128 i, {int,h0,hL}*64+o]
    mwT_sb = ctile([128, 192], F32R, "mwT_sb")
    nc.sync.dma_start(mwT_sb[:, :], io["mwT"])
    wsumT = {"int": mwT_sb[:, 0:64], "h0": mwT_sb[:, 64:128], "hL": mwT_sb[:, 128:192]}

    # pair-max tables: pm[t0, t1*512+ch] = max(emb[t0,ch], emb[t1,ch]).
    # Built as two partition-base-0 pieces (t0 0..7 / 8..13), then reshaped
    # to [pair, ch] partition chunks by SBUF->SBUF DMA (all on-chip).
    # Transients (flat-broadcast emb + pm halves) live in a scoped pool.
    def pair_table(prep, emb_dram, dt, dma_eng, tag):
        embA = prep.tile([8, EMB], dt, tag="embA", name=tag + "_embA")
        nc.gpsimd.dma_start(embA[:, :], emb_dram[0:8, :])
        embB = prep.tile([6, EMB], dt, tag="embB", name=tag + "_embB")
        nc.gpsimd.dma_start(embB[:, :], emb_dram[8:V, :])
        embF = prep.tile([V, V * EMB], dt, tag="embF", name=tag + "_embF")
        nc.gpsimd.dma_start(
            embF[:, :],
            emb_dram.rearrange("v e -> () (v e)").partition_broadcast(V))
        tps = []
        for half, esb, nt0 in (("0", embA, 8), ("1", embB, 6)):
            pm = prep.tile([nt0, V * EMB], dt, tag="pm", name=tag + "pm" + half)
            for t1 in range(V):
                nc.vector.tensor_tensor(pm[:, t1 * EMB:(t1 + 1) * EMB],
                                        esb[:, :], embF[0:nt0, t1 * EMB:(t1 + 1) * EMB],
                                        ALU.max)
            tp = ctile([nt0 * V, EMB], dt, tag + half)
            nc.gpsimd.dma_start(tp[:, :], pm[:, :])
            tps.append(tp)
        return tps[0], tps[1]

    with tc.tile_pool(name="tblprep", bufs=1) as prep:
        tpE0, tpE1 = pair_table(prep, io["eemb"], F32R, nc.sync, "tpE")
        tpF0, tpF1 = pair_table(prep, io["femb"], BF16, nc.scalar, "tpF")

    # ---------------- shared stage helpers ----------------
    def embed_pool_grp(idx_row, g, tp0, tp1, slab_dt, tag):
        """Group g (8 samples): one-hot embed + pair-max -> 4 padded slabs."""
        slabs = [slabp.tile([128, SLABW], slab_dt, tag=f"slab{kc}",
                            name=f"{tag}slab{kc}_{g}") for kc in range(4)]
        zsrc = zpadb if slab_dt == BF16 else zpadr
        npad = SLAB + 1
        for kc in range(4):
            nc.vector.tensor_copy(slabs[kc][:, 0:SLABW:H + 1], zsrc[:, 0:npad])
        npos = SLAB * H  # 1024
        oh0 = ohp.tile([PAIR0, npos], slab_dt, tag="oh0", name=f"{tag}oh0_{g}")
        oh1 = ohp.tile([PAIR1, npos], slab_dt, tag="oh1", name=f"{tag}oh1_{g}")
        for nt in range(npos // 512):
            idxpp = psum_emb.tile([PAIR0, 512], F32, tag="pp", name="idxpp")
            nc.tensor.matmul(idxpp[:, :], ones_row[:, 0:PAIR0],
                             idx_row[:, g * npos + nt * 512:
                                     g * npos + (nt + 1) * 512],
                             start=True, stop=True)
            nc.vector.tensor_scalar(oh0[:, nt * 512:(nt + 1) * 512],
                                    idxpp[:, :], iota_col[0:PAIR0, :],
                                    None, ALU.is_equal)
            nc.vector.tensor_scalar(oh1[:, nt * 512:(nt + 1) * 512],
                                    idxpp[0:PAIR1, :], float(PAIR0),
                                    iota_col[0:PAIR1, :], ALU.subtract,
                                    ALU.is_equal)
        for kc in range(4):
            for nt in range(npos // 512):
                pp = psum_emb.tile([128, 512], F32, tag="pp", name="pp")
                nc.tensor.matmul(pp[:, :], tp0[:, kc * 128:(kc + 1) * 128],
                                 oh0[:, nt * 512:(nt + 1) * 512],
                                 start=True, stop=False)
                nc.tensor.matmul(pp[:, :], tp1[:, kc * 128:(kc + 1) * 128],
                                 oh1[:, nt * 512:(nt + 1) * 512],
                                 start=False, stop=True)
                # scatter 4 samples x 128 positions into the padded slab
                s0 = nt * 4
                dst = slabs[kc][:, 1 + s0 * (H + 1): 1 + (s0 + 4) * (H + 1)] \
                    .rearrange("p (s w) -> p s w", w=H + 1)[:, :, 0:H]
                nc.vector.tensor_copy(
                    dst, pp[:, :].rearrange("p (s w) -> p s w", w=H))
        return slabs

    def conv_grp(slabs, g, wts, bias_bc, acts):
        """3-tap conv for the 8 samples of group g; write biased acts."""
        for ls in range(SLAB):
            s = g * SLAB + ls
            cp = psum_conv.tile([128, 256], F32, tag="cp", name="cp")
            first = True
            for kc in range(4):
                for dh in range(3):
                    lhsT = slabs[kc][:, ls * (H + 1) + dh: ls * (H + 1) + dh + 128]
                    rhs = wts[kc][:, dh * 256:(dh + 1) * 256]
                    nc.tensor.matmul(cp[:, :], lhsT, rhs,
                                     start=first, stop=(kc == 3 and dh == 2))
                    first = False
            nc.vector.tensor_tensor(acts[:, s * 256:(s + 1) * 256],
                                    cp[:, :], bias_bc[:, :], ALU.add)

    def big_linear(acts, w_dram, wdt, pool, dma_eng, grp, rowk, tag):
        """psum[j(128), b(32)] = sum_c W_c^T @ acts[:, (b, o=c)].

        Streams W in batches of `grp` 128-row chunks per DMA (rowk chunks
        packed per DRAM row) through `pool` ring buffers on `dma_eng`.
        """
        lp = psum_lin.tile([128, BC], F32, tag="lp", name=f"{tag}_lp")
        nrows = grp // rowk * 128
        ncols = rowk * 128
        for g in range(256 // grp):
            wsb = pool.tile([128, grp * 128], wdt, tag="w", name=f"{tag}w{g}")
            src = w_dram[g * nrows:(g + 1) * nrows, :] \
                .rearrange("(j p) k -> p j k", p=128)
            dma_eng.dma_start(wsb[:, :].rearrange("p (j k) -> p j k", k=ncols), src)
            for jj in range(grp):
                c = g * grp + jj
                rhs = acts[:, c:c + (BC - 1) * 256 + 1:256]
                nc.tensor.matmul(lp[:, :], wsb[:, jj * 128:(jj + 1) * 128], rhs,
                                 start=(c == 0), stop=(c == 255))
        return lp

    # ---------------- enemy branch ----------------
    xsb = wtile([BC, L], I32, "xsb")
    nc.gpsimd.dma_start(xsb[:, :], io["x"])
    xf = wtile([BC, L], F32, "xf")
    nc.vector.tensor_copy(xf[:, :], xsb[:, :])
    idxE = wtile([BC, H], F32, "idxE")
    nc.vector.scalar_tensor_tensor(idxE[:, :], xf[:, 0:L:2], float(V),
                                   xf[:, 1:L:2], ALU.mult, ALU.add)
    idxrowE = wtile([1, BC * H], F32, "idxrow")
    nc.gpsimd.dma_start(idxrowE[:, :], idxE[:, :])

    actsE = wtile([128, BC * 256], F32, "acts")
    for g in range(NGRP):
        slabs = embed_pool_grp(idxrowE, g, tpE0, tpE1, F32R, "E")
        conv_grp(slabs, g, wtE, EBc, actsE)

    elw_pool = ctx.enter_context(tc.tile_pool(name="elw_pool", bufs=6))
    lpE = big_linear(actsE, io["elw"], F32, elw_pool, nc.sync, 8, 1, "E")
    # softmax over j (partition dim): exp, sum via matmul, normalize
    Ex = wtile([128, BC], F32, "Ex")
    nc.scalar.activation(Ex[:, :], lpE[:, :], AF.Exp, bias=elb_col[:, :])
    s1 = psum_sm.tile([BC, 1], F32, tag="sm", name="s1")
    nc.tensor.matmul(s1[:, :], Ex[:, :], ones_col[:, :], start=True, stop=True)
    r32 = wtile([BC, 1], F32, "r32")
    nc.vector.reciprocal(r32[:, :], s1[:, :])
    rrow = wtile([1, BC], F32, "rrow")
    nc.gpsimd.dma_start(rrow[:, :], r32[:, :])
    rbp = psum_sm.tile([128, BC], F32, tag="sm", name="rbp")
    nc.tensor.matmul(rbp[:, :], ones_row[:, :], rrow[:, :], start=True, stop=True)
    vT = wtile([128, BC], F32R, "vT")   # enemy_out^T [i, b]
    nc.vector.tensor_tensor(vT[:, :], Ex[:, :], rbp[:, :], ALU.mult)

    # ---------------- manipulator ----------------
    mlw_pool = ctx.enter_context(tc.tile_pool(name="mlw_pool", bufs=2))
    with tc.tile_pool(name="manip_rows", bufs=1) as mrows:
        rowsb = {}
        for name in ("int", "h0", "hL"):
            cx = psum_sm.tile([64, BC], F32, tag="sm", name="cx")
            nc.tensor.matmul(cx[:, :], wsumT[name],
                             vT[:, :], start=True, stop=True)
            cxs = mrows.tile([64, BC], F32, tag=f"cxs_{name}", name=f"cxs_{name}")
            nc.scalar.activation(cxs[:, :], cx[:, :], AF.Relu, bias=mcb_col[:, :])
            rowsb[name] = mrows.tile([1, 64 * BC], F32, tag=f"row_{name}",
                                     name=f"row_{name}")
            nc.gpsimd.dma_start(rowsb[name][:, :], cxs[:, :])
        # assemble [128 h, (o, b)] manip acts: rows 1..126 = interior variant,
        # row 0 = h0 variant, row 127 = hL variant, via K=1 mask matmuls
        acts_m = wtile([128, 64 * BC], F32R, "acts_m")
        for nt in range(64 * BC // 512):
            amp = psum_emb.tile([128, 512], F32, tag="pp", name="amp")
            sl = slice(nt * 512, (nt + 1) * 512)
            nc.tensor.matmul(amp[:, :], ei_row[:, :], rowsb["int"][:, sl],
                             start=True, stop=False)
            nc.tensor.matmul(amp[:, :], e0_row[:, :], rowsb["h0"][:, sl],
                             start=False, stop=False)
            nc.tensor.matmul(amp[:, :], eL_row[:, :], rowsb["hL"][:, sl],
                             start=False, stop=True)
            nc.vector.tensor_copy(acts_m[:, sl], amp[:, :])

    mp = psum_lin.tile([BC, 256], F32, tag="lp", name="mp")
    for g in range(8):
        wsb = mlw_pool.tile([128, 8 * 256], F32R, tag="w", name=f"mw{g}")
        src = io["mlw"][g * 1024:(g + 1) * 1024, :] \
            .rearrange("(j p) k -> p j k", p=128)
        nc.scalar.dma_start(wsb[:, :].rearrange("p (j k) -> p j k", k=256), src)
        for jj in range(8):
            c = g * 8 + jj
            nc.tensor.matmul(mp[:, :], acts_m[:, c * BC:(c + 1) * BC],
                             wsb[:, jj * 256:(jj + 1) * 256],
                             start=(c == 0), stop=(c == 63))
    m_sb = wtile([BC, 256], F32, "m_sb")
    nc.vector.tensor_tensor(m_sb[:, :], mp[:, :], MBc[0:BC, :], ALU.add)

    # tokens = floor(|m|*100) mod 14; pair idx = 14*even + odd
    # floor via the 2^23 magic-number trick (t in [0, ~50) << 2^23):
    #   round_nearest(t - 0.5 + 2^23) - 2^23 == floor(t) for non-integer t
    # mod 14 via repeated conditional subtract (covers t < 42)
    tt = wtile([BC, 256], F32, "tt")
    nc.scalar.activation(tt[:, :], m_sb[:, :], AF.Abs, scale=100.0)
    fu = wtile([BC, 256], F32, "fu")
    nc.vector.tensor_scalar(fu[:, :], tt[:, :], 8388607.5, None, ALU.add)
    fr = wtile([BC, 256], F32, "fr")
    nc.vector.tensor_scalar(fr[:, :], fu[:, :], 8388608.0, None, ALU.subtract)
    ti = wtile([BC, 256], F32, "ti")
    nc.vector.tensor_scalar(ti[:, :], fr[:, :], float(V), None, ALU.is_ge)
    t1 = wtile([BC, 256], F32, "t1")
    nc.vector.scalar_tensor_tensor(t1[:, :], ti[:, :], -float(V), fr[:, :],
                                   ALU.mult, ALU.add)
    t2 = wtile([BC, 256], F32, "t2")
    nc.vector.tensor_scalar(t2[:, :], t1[:, :], float(V), None, ALU.is_ge)
    tok = wtile([BC, 256], F32, "tok")
    nc.vector.scalar_tensor_tensor(tok[:, :], t2[:, :], -float(V), t1[:, :],
                                   ALU.mult, ALU.add)
    idxF = wtile([BC, H], F32, "idxF")
    nc.vector.scalar_tensor_tensor(idxF[:, :], tok[:, 0:256:2], float(V),
                                   tok[:, 1:256:2], ALU.mult, ALU.add)
    idxrowF = wtile([1, BC * H], F32, "idxrow")
    nc.gpsimd.dma_start(idxrowF[:, :], idxF[:, :])

    # ---------------- friend branch (bf16) ----------------
    flw_pool = ctx.enter_context(tc.tile_pool(name="flw_pool", bufs=4))
    actsF = wtile([128, BC * 256], BF16, "acts")
    for g in range(NGRP):
        slabs = embed_pool_grp(idxrowF, g, tpF0, tpF1, BF16, "F")
        conv_grp(slabs, g, wtF, FBc, actsF)

    lpF = big_linear(actsF, io["flw2"], BF16, flw_pool, nc.scalar, 16, 2, "F")
    fsb = wtile([128, BC], F32, "fsb")
    nc.vector.tensor_scalar(fsb[:, :], lpF[:, :], flb_col[:, :], None, ALU.add)

    w2sb = wtile([128, 14], F32, "w2sb")
    nc.gpsimd.dma_start(w2sb[:, :], io["f2w"])
    f2 = psum_sm.tile([BC, 14], F32, tag="sm", name="f2")
    nc.tensor.matmul(f2[:, :], fsb[:, :], w2sb[:, :], start=True, stop=True)
    logits = wtile([BC, 14], F32, "logits")
    nc.vector.tensor_tensor(logits[:, :], f2[:, :], F2Bc[0:BC, :], ALU.add)
    nmx = wtile([BC, 1], F32, "nmx")
    nc.vector.reduce_max(nmx[:, :], logits[:, :], AX.X, negate=True)
    ex = wtile([BC, 14], F32, "ex")
    nc.scalar.activation(ex[:, :], logits[:, :], AF.Exp, bias=nmx[:, :])
    sm = wtile([BC, 1], F32, "sm")
    nc.vector.reduce_sum(sm[:, :], ex[:, :], AX.X)
    rs = wtile([BC, 1], F32, "rs")
    nc.vector.reciprocal(rs[:, :], sm[:, :])
    outt = wtile([BC, 14], F32, "outt")
    nc.vector.tensor_scalar(outt[:, :], ex[:, :], rs[:, :], None, ALU.mult)
    nc.gpsimd.dma_start(io["out"], outt[:, :])


_CACHE = {}


def _get_nc():
    if "nc" not in _CACHE:
        nc = bacc.Bacc("TRN2", target_bir_lowering=False, debug=False,
                       num_devices=NCORES)
        with tile.TileContext(nc) as tc:
            with ExitStack() as ctx:
                build_kernel(nc, tc, ctx)
        nc.compile()
        _CACHE["nc"] = nc
    return _CACHE["nc"]


def prep_inputs(inputs):
    """Host-side shard/layout prep. Returns list of 8 in_maps."""
    f32 = np.float32
    bf16 = ml_dtypes.bfloat16

    ecw = np.asarray(inputs["enemy_conv_w"], f32)[:, :, :, 1]   # [256 o,512 i,3]
    ecwT = np.ascontiguousarray(ecw.transpose(1, 2, 0)).reshape(512, 768)
    fcw = np.asarray(inputs["friend_conv_w"], f32)[:, :, :, 1]
    fcwT = np.ascontiguousarray(fcw.transpose(1, 2, 0)).reshape(512, 768) \
        .astype(bf16)
    mcw = np.asarray(inputs["manip_conv_w"], f32)[:, :, :, 1]   # [64 o,128 i,3]
    m_int = (mcw[:, :, 0] + mcw[:, :, 1] + mcw[:, :, 2]).T      # [128 i, 64 o]
    m_h0 = (mcw[:, :, 1] + mcw[:, :, 2]).T
    m_hL = (mcw[:, :, 0] + mcw[:, :, 1]).T
    mwT = np.ascontiguousarray(np.concatenate([m_int, m_h0, m_hL], axis=1))
    flw = np.asarray(inputs["friend_lin1_w"], f32)              # [32768, 128]
    flw2 = np.ascontiguousarray(
        flw.reshape(128, 2, 128, 128).transpose(0, 2, 1, 3)
        .reshape(16384, 256)).astype(bf16)

    common = {
        "eemb": np.ascontiguousarray(inputs["enemy_emb"], f32),
        "ecwT": ecwT,
        "ecb": np.ascontiguousarray(inputs["enemy_conv_b"], f32),
        "elw": np.ascontiguousarray(inputs["enemy_lin_w"], f32),
        "elb": np.ascontiguousarray(inputs["enemy_lin_b"], f32),
        "mwT": mwT,
        "mcb": np.ascontiguousarray(inputs["manip_conv_b"], f32),
        "mlw": np.ascontiguousarray(inputs["manip_lin_w"], f32),
        "mlb": np.ascontiguousarray(inputs["manip_lin_b"], f32),
        "femb": np.asarray(inputs["friend_emb"]).astype(bf16),
        "fcwT": fcwT,
        "fcb": np.ascontiguousarray(inputs["friend_conv_b"], f32),
        "flw2": flw2,
        "flb": np.ascontiguousarray(inputs["friend_lin1_b"], f32),
        "f2w": np.ascontiguousarray(inputs["friend_lin2_w"], f32),
        "f2b": np.ascontiguousarray(inputs["friend_lin2_b"], f32),
    }
    x = np.ascontiguousarray(inputs["x"], np.int32)
    return [dict(common, x=np.ascontiguousarray(x[c * BC:(c + 1) * BC]))
            for c in range(NCORES)]


def kernel(**inputs):
    nc = _get_nc()
    in_maps = prep_inputs(inputs)
    res = run_bass_kernel_spmd(nc, in_maps, core_ids=list(range(NCORES)))
    return np.concatenate([r["out"] for r in res.results], axis=0)


# revision 15
# speedup vs baseline: 2.3955x; 1.3458x over previous
"""Trainium2 Bass kernel for nn_Network_67388036874689.

Data-parallel over batch: B=256 sharded as 32 samples on each of 8 cores;
all parameters replicated.

Structure exploited (validated against the reference on host):
  - fog_of_war's greedy scan returns arange(B) -> the permutation is identity.
  - conv2d(3x3, pad=1) on [C, H, 1] spatial input only sees kernel column 1
    -> 1D conv over H with 3 taps.
  - Embedding lookup (V=14) followed by pair-maxpool = lookup into a 196-entry
    pairwise-max table, implemented as one-hot matmuls on the PE.
  - The manipulator conv input is constant over H -> collapses to 3 matmuls
    (interior / h=0 / h=127 tap-sum variants).

Performance structure:
  - Conv / tap-sum weight transposes are done host-side in prep_inputs.
  - The three big weight streams (elw 16.8MB, mlw 8.4MB, flw 8.4MB) are
    DMA'd in large batched transfers on the two HWDGE queues (sync carries
    elw, scalar carries mlw then flw) with ring buffers, so they prefetch
    underneath the conv phases instead of gating the linear phases.
  - friend_lin1_w is pair-packed host-side so every DMA descriptor is 512B.

Precision: critical path to the token discretization (enemy branch + manip)
in fp32 / float32r; post-token friend branch in bf16.
"""

import numpy as np
import ml_dtypes
from contextlib import ExitStack

import concourse.bass as bass
import concourse.bacc as bacc
import concourse.mybir as mybir
import concourse.tile as tile
from concourse.bass_utils import run_bass_kernel_spmd

F32 = mybir.dt.float32
F32R = mybir.dt.float32r
BF16 = mybir.dt.bfloat16
I32 = mybir.dt.int32
AF = mybir.ActivationFunctionType
ALU = mybir.AluOpType
AX = mybir.AxisListType

NCORES = 8
B = 256
BC = B // NCORES        # 32 samples per core
L = 256                 # sequence length
V = 14                  # vocab
EMB = 512               # embedding dim
H = L // 2              # 128 pooled positions
NPAIR = V * V           # 196
PAIR0 = 112             # pair-table chunk split: 112 (t0 0..7) + 84 (t0 8..13)
PAIR1 = NPAIR - PAIR0   # 84
SLAB = 8                # samples per embed/pool slab group
NGRP = BC // SLAB       # 4 groups
SLABW = SLAB * (H + 1) + 1   # padded slab width (stride 129 per sample)


def _dram_inputs(nc):
    t = {}

    def inp(name, shape, dt):
        t[name] = nc.dram_tensor(name, list(shape), dt, kind="ExternalInput").ap()

    inp("x", (BC, L), I32)
    inp("eemb", (V, EMB), BF16)
    inp("ecwT", (512, 3 * 256), BF16)     # [i_global, dh*256+o]
    inp("ecb", (256,), F32)
    inp("elw2", (16384, 256), BF16)       # pair-packed enemy_lin_w
    inp("elb", (128,), F32)
    inp("mwT", (128, 3 * 64), F32R)       # [i, {int,h0,hL}*64+o] tap sums
    inp("mcb", (64,), F32)
    inp("mlw", (8192, 256), F32R)
    inp("mlb", (256,), F32)
    inp("femb", (V, EMB), BF16)
    inp("fcwT", (512, 3 * 256), BF16)     # [i_global, dh*256+o]
    inp("fcb", (256,), F32)
    inp("flw2", (16384, 256), BF16)       # pair-packed friend_lin1_w
    inp("flb", (128,), F32)
    inp("f2w", (128, 14), F32)
    inp("f2b", (14,), F32)
    t["out"] = nc.dram_tensor("out", [BC, 14], F32, kind="ExternalOutput").ap()
    return t


def build_kernel(nc, tc, ctx):
    io = _dram_inputs(nc)
    consts = ctx.enter_context(tc.tile_pool(name="consts", bufs=1))
    work = ctx.enter_context(tc.tile_pool(name="work", bufs=1))
    slabp = ctx.enter_context(tc.tile_pool(name="slabp", bufs=2))
    ohp = ctx.enter_context(tc.tile_pool(name="ohp", bufs=1))
    psum_emb = ctx.enter_context(tc.tile_pool(name="psum_emb", bufs=4, space="PSUM"))
    psum_conv = ctx.enter_context(tc.tile_pool(name="psum_conv", bufs=2, space="PSUM"))
    psum_lin = ctx.enter_context(tc.tile_pool(name="psum_lin", bufs=1, space="PSUM"))
    psum_sm = ctx.enter_context(tc.tile_pool(name="psum_sm", bufs=1, space="PSUM"))

    def ctile(shape, dt, tag):
        return consts.tile(shape, dt, tag=tag, name=tag)

    def wtile(shape, dt, tag):
        return work.tile(shape, dt, tag=tag, name=tag)

    # ---------------- constants ----------------
    iota_i = ctile([128, 1], I32, "iota_i")
    nc.gpsimd.iota(iota_i[:, :], pattern=[[0, 1]], base=0, channel_multiplier=1)
    iota_col = ctile([128, 1], F32, "iota_col")
    nc.vector.tensor_copy(iota_col[:, :], iota_i[:, :])
    ones_col = ctile([128, 1], F32, "ones_col")
    nc.vector.memset(ones_col[:, :], 1.0)
    ones_row = ctile([1, 128], F32, "ones_row")
    nc.vector.memset(ones_row[:, :], 1.0)
    iota_row = ctile([1, 128], F32, "iota_row")
    nc.gpsimd.dma_start(iota_row[:, :], iota_col[:, :])
    e0_row = ctile([1, 128], F32, "e0_row")
    nc.vector.tensor_scalar(e0_row[:, :], iota_row[:, :], 0.0, None, ALU.is_equal)
    eL_row = ctile([1, 128], F32, "eL_row")
    nc.vector.tensor_scalar(eL_row[:, :], iota_row[:, :], 127.0, None, ALU.is_equal)
    ei_row = ctile([1, 128], F32, "ei_row")
    nc.vector.scalar_tensor_tensor(ei_row[:, :], e0_row[:, :], -1.0, eL_row[:, :],
                                   ALU.mult, ALU.subtract)
    nc.vector.tensor_scalar(ei_row[:, :], ei_row[:, :], 1.0, None, ALU.add)
    zpadb = ctile([128, 32], BF16, "zpadb")
    nc.vector.memset(zpadb[:, :], 0.0)

    def bias_col(dram_vec, n, tag):
        col = ctile([n, 1], F32, tag)
        nc.gpsimd.dma_start(col[:, :], dram_vec)
        return col

    def bias_bcast(dram_vec, rows, width, tag):
        out = ctile([rows, width], F32, tag)
        nc.gpsimd.dma_start(out[:, :], dram_vec[None, :].partition_broadcast(rows))
        return out

    EBc = bias_bcast(io["ecb"], 128, 256, "EB")
    FBc = bias_bcast(io["fcb"], 128, 256, "FB")
    MBc = bias_bcast(io["mlb"], BC, 256, "MB")
    F2Bc = bias_bcast(io["f2b"], BC, 14, "F2B")
    elb_col = bias_col(io["elb"], 128, "elb")
    flb_col = bias_col(io["flb"], 128, "flb")
    mcb_col = bias_col(io["mcb"], 64, "mcb")

    # conv weights, already transposed host-side: 4 tiles [128 i, dh*256+o]
    wtE_all = ctile([128, 4 * 768], BF16, "wtE_all")
    nc.sync.dma_start(wtE_all[:, :].rearrange("p (kc d) -> p kc d", d=768),
                      io["ecwT"].rearrange("(kc p) d -> p kc d", p=128))
    wtE = [wtE_all[:, kc * 768:(kc + 1) * 768] for kc in range(4)]
    wtF_all = ctile([128, 4 * 768], BF16, "wtF_all")
    nc.scalar.dma_start(wtF_all[:, :].rearrange("p (kc d) -> p kc d", d=768),
                        io["fcwT"].rearrange("(kc p) d -> p kc d", p=128))
    wtF = [wtF_all[:, kc * 768:(kc + 1) * 768] for kc in range(4)]

    # manip tap-sum weights, host-transposed: [128 i, {int,h0,hL}*64+o]
    mwT_sb = ctile([128, 192], F32R, "mwT_sb")
    nc.sync.dma_start(mwT_sb[:, :], io["mwT"])
    wsumT = {"int": mwT_sb[:, 0:64], "h0": mwT_sb[:, 64:128], "hL": mwT_sb[:, 128:192]}

    # pair-max tables: pm[t0, t1*512+ch] = max(emb[t0,ch], emb[t1,ch]).
    # Built as two partition-base-0 pieces (t0 0..7 / 8..13), then reshaped
    # to [pair, ch] partition chunks by SBUF->SBUF DMA (all on-chip).
    # Transients (flat-broadcast emb + pm halves) live in a scoped pool.
    def pair_table(prep, emb_dram, dt, dma_eng, tag):
        embA = prep.tile([8, EMB], dt, tag="embA", name=tag + "_embA")
        nc.gpsimd.dma_start(embA[:, :], emb_dram[0:8, :])
        embB = prep.tile([6, EMB], dt, tag="embB", name=tag + "_embB")
        nc.gpsimd.dma_start(embB[:, :], emb_dram[8:V, :])
        embF = prep.tile([V, V * EMB], dt, tag="embF", name=tag + "_embF")
        nc.gpsimd.dma_start(
            embF[:, :],
            emb_dram.rearrange("v e -> () (v e)").partition_broadcast(V))
        tps = []
        for half, esb, nt0 in (("0", embA, 8), ("1", embB, 6)):
            pm = prep.tile([nt0, V * EMB], dt, tag="pm", name=tag + "pm" + half)
            for t1 in range(V):
                nc.vector.tensor_tensor(pm[:, t1 * EMB:(t1 + 1) * EMB],
                                        esb[:, :], embF[0:nt0, t1 * EMB:(t1 + 1) * EMB],
                                        ALU.max)
            tp = ctile([nt0 * V, EMB], dt, tag + half)
            nc.gpsimd.dma_start(tp[:, :], pm[:, :])
            tps.append(tp)
        return tps[0], tps[1]

    # ---------------- shared stage helpers ----------------
    def embed_pool_grp(idx_row, g, tp0, tp1, slab_dt, tag):
        """Group g (8 samples): one-hot embed + pair-max -> 4 padded slabs."""
        slabs = [slabp.tile([128, SLABW], slab_dt, tag=f"slab{kc}",
                            name=f"{tag}slab{kc}_{g}") for kc in range(4)]
        npad = SLAB + 1
        for kc in range(4):
            nc.vector.tensor_copy(slabs[kc][:, 0:SLABW:H + 1], zpadb[:, 0:npad])
        npos = SLAB * H  # 1024
        oh0 = ohp.tile([PAIR0, npos], slab_dt, tag="oh0", name=f"{tag}oh0_{g}")
        oh1 = ohp.tile([PAIR1, npos], slab_dt, tag="oh1", name=f"{tag}oh1_{g}")
        for nt in range(npos // 512):
            idxpp = psum_emb.tile([PAIR0, 512], F32, tag="pp", name="idxpp")
            nc.tensor.matmul(idxpp[:, :], ones_row[:, 0:PAIR0],
                             idx_row[:, g * npos + nt * 512:
                                     g * npos + (nt + 1) * 512],
                             start=True, stop=True)
            nc.vector.tensor_scalar(oh0[:, nt * 512:(nt + 1) * 512],
                                    idxpp[:, :], iota_col[0:PAIR0, :],
                                    None, ALU.is_equal)
            nc.vector.tensor_scalar(oh1[:, nt * 512:(nt + 1) * 512],
                                    idxpp[0:PAIR1, :], float(PAIR0),
                                    iota_col[0:PAIR1, :], ALU.subtract,
                                    ALU.is_equal)
        for kc in range(4):
            for nt in range(npos // 512):
                pp = psum_emb.tile([128, 512], F32, tag="pp", name="pp")
                nc.tensor.matmul(pp[:, :], tp0[:, kc * 128:(kc + 1) * 128],
                                 oh0[:, nt * 512:(nt + 1) * 512],
                                 start=True, stop=False)
                nc.tensor.matmul(pp[:, :], tp1[:, kc * 128:(kc + 1) * 128],
                                 oh1[:, nt * 512:(nt + 1) * 512],
                                 start=False, stop=True)
                # scatter 4 samples x 128 positions into the padded slab
                s0 = nt * 4
                dst = slabs[kc][:, 1 + s0 * (H + 1): 1 + (s0 + 4) * (H + 1)] \
                    .rearrange("p (s w) -> p s w", w=H + 1)[:, :, 0:H]
                nc.vector.tensor_copy(
                    dst, pp[:, :].rearrange("p (s w) -> p s w", w=H))
        return slabs

    def conv_grp(slabs, g, wts, bias_bc, acts):
        """3-tap conv for the 8 samples of group g; write biased acts."""
        for ls in range(SLAB):
            s = g * SLAB + ls
            cp = psum_conv.tile([128, 256], F32, tag="cp", name="cp")
            first = True
            for kc in range(4):
                for dh in range(3):
                    lhsT = slabs[kc][:, ls * (H + 1) + dh: ls * (H + 1) + dh + 128]
                    rhs = wts[kc][:, dh * 256:(dh + 1) * 256]
                    nc.tensor.matmul(cp[:, :], lhsT, rhs,
                                     start=first, stop=(kc == 3 and dh == 2))
                    first = False
            nc.vector.tensor_tensor(acts[:, s * 256:(s + 1) * 256],
                                    cp[:, :], bias_bc[:, :], ALU.add)

    def big_linear(acts, w_dram, wdt, pool, dma_eng, grp, rowk, tag):
        """psum[j(128), b(32)] = sum_c W_c^T @ acts[:, (b, o=c)].

        Streams W in batches of `grp` 128-row chunks per DMA (rowk chunks
        packed per DRAM row) through `pool` ring buffers on `dma_eng`.
        """
        lp = psum_lin.tile([128, BC], F32, tag="lp", name=f"{tag}_lp")
        nrows = grp // rowk * 128
        ncols = rowk * 128
        for g in range(256 // grp):
            wsb = pool.tile([128, grp * 128], wdt, tag="w", name=f"{tag}w{g}")
            src = w_dram[g * nrows:(g + 1) * nrows, :] \
                .rearrange("(j p) k -> p j k", p=128)
            dma_eng.dma_start(wsb[:, :].rearrange("p (j k) -> p j k", k=ncols), src)
            for jj in range(grp):
                c = g * grp + jj
                rhs = acts[:, c:c + (BC - 1) * 256 + 1:256]
                nc.tensor.matmul(lp[:, :], wsb[:, jj * 128:(jj + 1) * 128], rhs,
                                 start=(c == 0), stop=(c == 255))
        return lp

    # ---------------- enemy branch ----------------
    xsb = wtile([BC, L], I32, "xsb")
    nc.gpsimd.dma_start(xsb[:, :], io["x"])
    xf = wtile([BC, L], F32, "xf")
    nc.vector.tensor_copy(xf[:, :], xsb[:, :])
    idxE = wtile([BC, H], F32, "idxE")
    nc.vector.scalar_tensor_tensor(idxE[:, :], xf[:, 0:L:2], float(V),
                                   xf[:, 1:L:2], ALU.mult, ALU.add)
    idxrowE = wtile([1, BC * H], F32, "idxrow")
    nc.gpsimd.dma_start(idxrowE[:, :], idxE[:, :])

    actsE = wtile([128, BC * 256], BF16, "acts")
    with tc.tile_pool(name="tblprep", bufs=1) as prep:
        tpE0, tpE1 = pair_table(prep, io["eemb"], BF16, nc.sync, "tpE")
        for g in range(NGRP):
            slabs = embed_pool_grp(idxrowE, g, tpE0, tpE1, BF16, "E")
            conv_grp(slabs, g, wtE, EBc, actsE)
        # friend tables build here: their DMA/vector work hides under the
        # enemy conv phase, and the prep buffers are reused in place.
        tpF0, tpF1 = pair_table(prep, io["femb"], BF16, nc.scalar, "tpF")

    elw_pool = ctx.enter_context(tc.tile_pool(name="elw_pool", bufs=6))
    lpE = big_linear(actsE, io["elw2"], BF16, elw_pool, nc.sync, 16, 2, "E")
    # softmax over j (partition dim): exp, sum via matmul, normalize
    Ex = wtile([128, BC], F32, "Ex")
    nc.scalar.activation(Ex[:, :], lpE[:, :], AF.Exp, bias=elb_col[:, :])
    s1 = psum_sm.tile([BC, 1], F32, tag="sm", name="s1")
    nc.tensor.matmul(s1[:, :], Ex[:, :], ones_col[:, :], start=True, stop=True)
    r32 = wtile([BC, 1], F32, "r32")
    nc.vector.reciprocal(r32[:, :], s1[:, :])
    rrow = wtile([1, BC], F32, "rrow")
    nc.gpsimd.dma_start(rrow[:, :], r32[:, :])
    rbp = psum_sm.tile([128, BC], F32, tag="sm", name="rbp")
    nc.tensor.matmul(rbp[:, :], ones_row[:, :], rrow[:, :], start=True, stop=True)
    vT = wtile([128, BC], F32R, "vT")   # enemy_out^T [i, b]
    nc.vector.tensor_tensor(vT[:, :], Ex[:, :], rbp[:, :], ALU.mult)

    # ---------------- manipulator ----------------
    mlw_pool = ctx.enter_context(tc.tile_pool(name="mlw_pool", bufs=2))
    with tc.tile_pool(name="manip_rows", bufs=1) as mrows:
        rowsb = {}
        for name in ("int", "h0", "hL"):
            cx = psum_sm.tile([64, BC], F32, tag="sm", name="cx")
            nc.tensor.matmul(cx[:, :], wsumT[name],
                             vT[:, :], start=True, stop=True)
            cxs = mrows.tile([64, BC], F32, tag=f"cxs_{name}", name=f"cxs_{name}")
            nc.scalar.activation(cxs[:, :], cx[:, :], AF.Relu, bias=mcb_col[:, :])
            rowsb[name] = mrows.tile([1, 64 * BC], F32, tag=f"row_{name}",
                                     name=f"row_{name}")
            nc.gpsimd.dma_start(rowsb[name][:, :], cxs[:, :])
        # assemble [128 h, (o, b)] manip acts: rows 1..126 = interior variant,
        # row 0 = h0 variant, row 127 = hL variant, via K=1 mask matmuls
        acts_m = wtile([128, 64 * BC], F32R, "acts_m")
        for nt in range(64 * BC // 512):
            amp = psum_emb.tile([128, 512], F32, tag="pp", name="amp")
            sl = slice(nt * 512, (nt + 1) * 512)
            nc.tensor.matmul(amp[:, :], ei_row[:, :], rowsb["int"][:, sl],
                             start=True, stop=False)
            nc.tensor.matmul(amp[:, :], e0_row[:, :], rowsb["h0"][:, sl],
                             start=False, stop=False)
            nc.tensor.matmul(amp[:, :], eL_row[:, :], rowsb["hL"][:, sl],
                             start=False, stop=True)
            nc.vector.tensor_copy(acts_m[:, sl], amp[:, :])

    mp = psum_lin.tile([BC, 256], F32, tag="lp", name="mp")
    for g in range(8):
        wsb = mlw_pool.tile([128, 8 * 256], F32R, tag="w", name=f"mw{g}")
        src = io["mlw"][g * 1024:(g + 1) * 1024, :] \
            .rearrange("(j p) k -> p j k", p=128)
        nc.scalar.dma_start(wsb[:, :].rearrange("p (j k) -> p j k", k=256), src)
        for jj in range(8):
            c = g * 8 + jj
            nc.tensor.matmul(mp[:, :], acts_m[:, c * BC:(c + 1) * BC],
                             wsb[:, jj * 256:(jj + 1) * 256],
                             start=(c == 0), stop=(c == 63))
    m_sb = wtile([BC, 256], F32, "m_sb")
    nc.vector.tensor_tensor(m_sb[:, :], mp[:, :], MBc[0:BC, :], ALU.add)

    # tokens = floor(|m|*100) mod 14; pair idx = 14*even + odd
    # floor via the 2^23 magic-number trick (t in [0, ~50) << 2^23):
    #   round_nearest(t - 0.5 + 2^23) - 2^23 == floor(t) for non-integer t
    # mod 14 via repeated conditional subtract (covers t < 42)
    tt = wtile([BC, 256], F32, "tt")
    nc.scalar.activation(tt[:, :], m_sb[:, :], AF.Abs, scale=100.0)
    fu = wtile([BC, 256], F32, "fu")
    nc.vector.tensor_scalar(fu[:, :], tt[:, :], 8388607.5, None, ALU.add)
    fr = wtile([BC, 256], F32, "fr")
    nc.vector.tensor_scalar(fr[:, :], fu[:, :], 8388608.0, None, ALU.subtract)
    ti = wtile([BC, 256], F32, "ti")
    nc.vector.tensor_scalar(ti[:, :], fr[:, :], float(V), None, ALU.is_ge)
    t1 = wtile([BC, 256], F32, "t1")
    nc.vector.scalar_tensor_tensor(t1[:, :], ti[:, :], -float(V), fr[:, :],
                                   ALU.mult, ALU.add)
    t2 = wtile([BC, 256], F32, "t2")
    nc.vector.tensor_scalar(t2[:, :], t1[:, :], float(V), None, ALU.is_ge)
    tok = wtile([BC, 256], F32, "tok")
    nc.vector.scalar_tensor_tensor(tok[:, :], t2[:, :], -float(V), t1[:, :],
                                   ALU.mult, ALU.add)
    idxF = wtile([BC, H], F32, "idxF")
    nc.vector.scalar_tensor_tensor(idxF[:, :], tok[:, 0:256:2], float(V),
                                   tok[:, 1:256:2], ALU.mult, ALU.add)
    idxrowF = wtile([1, BC * H], F32, "idxrow")
    nc.gpsimd.dma_start(idxrowF[:, :], idxF[:, :])

    # ---------------- friend branch (bf16) ----------------
    flw_pool = ctx.enter_context(tc.tile_pool(name="flw_pool", bufs=4))
    actsF = wtile([128, BC * 256], BF16, "acts")
    for g in range(NGRP):
        slabs = embed_pool_grp(idxrowF, g, tpF0, tpF1, BF16, "F")
        conv_grp(slabs, g, wtF, FBc, actsF)

    lpF = big_linear(actsF, io["flw2"], BF16, flw_pool, nc.scalar, 16, 2, "F")
    fsb = wtile([128, BC], F32, "fsb")
    nc.vector.tensor_scalar(fsb[:, :], lpF[:, :], flb_col[:, :], None, ALU.add)

    w2sb = wtile([128, 14], F32, "w2sb")
    nc.gpsimd.dma_start(w2sb[:, :], io["f2w"])
    f2 = psum_sm.tile([BC, 14], F32, tag="sm", name="f2")
    nc.tensor.matmul(f2[:, :], fsb[:, :], w2sb[:, :], start=True, stop=True)
    logits = wtile([BC, 14], F32, "logits")
    nc.vector.tensor_tensor(logits[:, :], f2[:, :], F2Bc[0:BC, :], ALU.add)
    nmx = wtile([BC, 1], F32, "nmx")
    nc.vector.reduce_max(nmx[:, :], logits[:, :], AX.X, negate=True)
    ex = wtile([BC, 14], F32, "ex")
    nc.scalar.activation(ex[:, :], logits[:, :], AF.Exp, bias=nmx[:, :])
    sm = wtile([BC, 1], F32, "sm")
    nc.vector.reduce_sum(sm[:, :], ex[:, :], AX.X)
    rs = wtile([BC, 1], F32, "rs")
    nc.vector.reciprocal(rs[:, :], sm[:, :])
    outt = wtile([BC, 14], F32, "outt")
    nc.vector.tensor_scalar(outt[:, :], ex[:, :], rs[:, :], None, ALU.mult)
    nc.gpsimd.dma_start(io["out"], outt[:, :])


_CACHE = {}


def _get_nc():
    if "nc" not in _CACHE:
        nc = bacc.Bacc("TRN2", target_bir_lowering=False, debug=False,
                       num_devices=NCORES)
        with tile.TileContext(nc) as tc:
            with ExitStack() as ctx:
                build_kernel(nc, tc, ctx)
        nc.compile()
        _CACHE["nc"] = nc
    return _CACHE["nc"]


def prep_inputs(inputs):
    """Host-side shard/layout prep. Returns list of 8 in_maps."""
    f32 = np.float32
    bf16 = ml_dtypes.bfloat16

    ecw = np.asarray(inputs["enemy_conv_w"], f32)[:, :, :, 1]   # [256 o,512 i,3]
    ecwT = np.ascontiguousarray(ecw.transpose(1, 2, 0)).reshape(512, 768) \
        .astype(bf16)
    fcw = np.asarray(inputs["friend_conv_w"], f32)[:, :, :, 1]
    fcwT = np.ascontiguousarray(fcw.transpose(1, 2, 0)).reshape(512, 768) \
        .astype(bf16)
    mcw = np.asarray(inputs["manip_conv_w"], f32)[:, :, :, 1]   # [64 o,128 i,3]
    m_int = (mcw[:, :, 0] + mcw[:, :, 1] + mcw[:, :, 2]).T      # [128 i, 64 o]
    m_h0 = (mcw[:, :, 1] + mcw[:, :, 2]).T
    m_hL = (mcw[:, :, 0] + mcw[:, :, 1]).T
    mwT = np.ascontiguousarray(np.concatenate([m_int, m_h0, m_hL], axis=1))
    def pack2(w):  # [32768,128] -> pair-packed [16384,256]
        return np.ascontiguousarray(
            np.asarray(w, f32).reshape(128, 2, 128, 128).transpose(0, 2, 1, 3)
            .reshape(16384, 256)).astype(bf16)

    common = {
        "eemb": np.asarray(inputs["enemy_emb"]).astype(bf16),
        "ecwT": ecwT,
        "ecb": np.ascontiguousarray(inputs["enemy_conv_b"], f32),
        "elw2": pack2(inputs["enemy_lin_w"]),
        "elb": np.ascontiguousarray(inputs["enemy_lin_b"], f32),
        "mwT": mwT,
        "mcb": np.ascontiguousarray(inputs["manip_conv_b"], f32),
        "mlw": np.ascontiguousarray(inputs["manip_lin_w"], f32),
        "mlb": np.ascontiguousarray(inputs["manip_lin_b"], f32),
        "femb": np.asarray(inputs["friend_emb"]).astype(bf16),
        "fcwT": fcwT,
        "fcb": np.ascontiguousarray(inputs["friend_conv_b"], f32),
        "flw2": pack2(inputs["friend_lin1_w"]),
        "flb": np.ascontiguousarray(inputs["friend_lin1_b"], f32),
        "f2w": np.ascontiguousarray(inputs["friend_lin2_w"], f32),
        "f2b": np.ascontiguousarray(inputs["friend_lin2_b"], f32),
    }
    x = np.ascontiguousarray(inputs["x"], np.int32)
    return [dict(common, x=np.ascontiguousarray(x[c * BC:(c + 1) * BC]))
            for c in range(NCORES)]


def kernel(**inputs):
    nc = _get_nc()
    in_maps = prep_inputs(inputs)
    res = run_bass_kernel_spmd(nc, in_maps, core_ids=list(range(NCORES)))
    return np.concatenate([r["out"] for r in res.results], axis=0)


# revision 32
# speedup vs baseline: 2.9505x; 1.2317x over previous
"""Trainium2 Bass kernel for nn_Network_67388036874689.

Data-parallel over batch: B=256 sharded as 32 samples on each of 8 cores;
all parameters replicated.

Structure exploited (validated against the reference on host):
  - fog_of_war's greedy scan returns arange(B) -> the permutation is identity.
  - conv2d(3x3, pad=1) on [C, H, 1] spatial input only sees kernel column 1
    -> 1D conv over H with 3 taps.
  - Embedding lookup (V=14) followed by pair-maxpool = lookup into a 196-entry
    pairwise-max table, implemented as one-hot matmuls on the PE.
  - The manipulator conv input is constant over H -> collapses to 3 matmuls
    (interior / h=0 / h=127 tap-sum variants).

Performance structure:
  - Conv / tap-sum weight transposes are done host-side in prep_inputs.
  - The three big weight streams (elw 16.8MB, mlw 8.4MB, flw 8.4MB) are
    DMA'd in large batched transfers on the two HWDGE queues (sync carries
    elw, scalar carries mlw then flw) with ring buffers, so they prefetch
    underneath the conv phases instead of gating the linear phases.
  - friend_lin1_w is pair-packed host-side so every DMA descriptor is 512B.

Precision: critical path to the token discretization (enemy branch + manip)
in fp32 / float32r; post-token friend branch in bf16.
"""

import numpy as np
import ml_dtypes
from contextlib import ExitStack

import concourse.bass as bass
import concourse.bacc as bacc
import concourse.mybir as mybir
import concourse.tile as tile
from concourse.bass_utils import run_bass_kernel_spmd

F32 = mybir.dt.float32
F32R = mybir.dt.float32r
BF16 = mybir.dt.bfloat16
I32 = mybir.dt.int32
AF = mybir.ActivationFunctionType
ALU = mybir.AluOpType
AX = mybir.AxisListType

NCORES = 8
B = 256
BC = B // NCORES        # 32 samples per core
L = 256                 # sequence length
V = 14                  # vocab
EMB = 512               # embedding dim
H = L // 2              # 128 pooled positions
NPAIR = V * V           # 196
PAIR0 = 112             # pair-table chunk split: 112 (t0 0..7) + 84 (t0 8..13)
PAIR1 = NPAIR - PAIR0   # 84
SLAB = 8                # samples per embed/pool slab group
NGRP = BC // SLAB       # 4 groups
SLABW = SLAB * (H + 1) + 1   # padded slab width (stride 129 per sample)


def _dram_inputs(nc):
    t = {}

    def inp(name, shape, dt):
        t[name] = nc.dram_tensor(name, list(shape), dt, kind="ExternalInput").ap()

    inp("x", (BC, L), I32)
    inp("eemb", (V, EMB), BF16)
    inp("ecwT", (512, 3 * 256), BF16)     # [i_global, dh*256+o]
    inp("ecb", (256,), F32)
    inp("elw2", (16384, 256), BF16)       # pair-packed enemy_lin_w
    inp("elb", (128,), F32)
    inp("mwT", (128, 3 * 64), F32R)       # [i, {int,h0,hL}*64+o] tap sums
    inp("mcb", (64,), F32)
    inp("mlws", (64, 3 * 256), F32R)
    inp("mlb", (256,), F32)
    inp("femb", (V, EMB), BF16)
    inp("fcwT", (512, 3 * 256), BF16)     # [i_global, dh*256+o]
    inp("fcb", (256,), F32)
    inp("flw2", (16384, 256), BF16)       # pair-packed friend_lin1_w
    inp("flb", (128,), F32)
    inp("f2w", (128, 14), F32)
    inp("f2b", (14,), F32)
    t["out"] = nc.dram_tensor("out", [BC, 14], F32, kind="ExternalOutput").ap()
    return t


def build_kernel(nc, tc, ctx):
    io = _dram_inputs(nc)
    consts = ctx.enter_context(tc.tile_pool(name="consts", bufs=1))
    work = ctx.enter_context(tc.tile_pool(name="work", bufs=1))
    slabp = ctx.enter_context(tc.tile_pool(name="slabp", bufs=2))
    ohp = ctx.enter_context(tc.tile_pool(name="ohp", bufs=1))
    psum_emb = ctx.enter_context(tc.tile_pool(name="psum_emb", bufs=4, space="PSUM"))
    psum_conv = ctx.enter_context(tc.tile_pool(name="psum_conv", bufs=2, space="PSUM"))
    psum_lin = ctx.enter_context(tc.tile_pool(name="psum_lin", bufs=1, space="PSUM"))
    psum_sm = ctx.enter_context(tc.tile_pool(name="psum_sm", bufs=1, space="PSUM"))

    def ctile(shape, dt, tag):
        return consts.tile(shape, dt, tag=tag, name=tag)

    def wtile(shape, dt, tag):
        return work.tile(shape, dt, tag=tag, name=tag)

    # ---------------- constants ----------------
    iota_i = ctile([128, 1], I32, "iota_i")
    nc.gpsimd.iota(iota_i[:, :], pattern=[[0, 1]], base=0, channel_multiplier=1)
    iota_col = ctile([128, 1], F32, "iota_col")
    nc.vector.tensor_copy(iota_col[:, :], iota_i[:, :])
    ones_col = ctile([128, 1], F32, "ones_col")
    nc.vector.memset(ones_col[:, :], 1.0)
    ones_row = ctile([1, 128], F32, "ones_row")
    nc.vector.memset(ones_row[:, :], 1.0)
    iota_row = ctile([1, 128], F32, "iota_row")
    nc.gpsimd.dma_start(iota_row[:, :], iota_col[:, :])
    e0_row = ctile([1, 128], F32, "e0_row")
    nc.vector.tensor_scalar(e0_row[:, :], iota_row[:, :], 0.0, None, ALU.is_equal)
    eL_row = ctile([1, 128], F32, "eL_row")
    nc.vector.tensor_scalar(eL_row[:, :], iota_row[:, :], 127.0, None, ALU.is_equal)
    ei_row = ctile([1, 128], F32, "ei_row")
    nc.vector.scalar_tensor_tensor(ei_row[:, :], e0_row[:, :], -1.0, eL_row[:, :],
                                   ALU.mult, ALU.subtract)
    nc.vector.tensor_scalar(ei_row[:, :], ei_row[:, :], 1.0, None, ALU.add)
    zpadb = ctile([128, 32], BF16, "zpadb")
    nc.vector.memset(zpadb[:, :], 0.0)

    def bias_col(dram_vec, n, tag):
        col = ctile([n, 1], F32, tag)
        nc.scalar.dma_start(col[:, :], dram_vec)
        return col

    def bias_bcast(dram_vec, rows, width, tag):
        out = ctile([rows, width], F32, tag)
        nc.gpsimd.dma_start(out[:, :], dram_vec[None, :].partition_broadcast(rows))
        return out

    EBc = bias_bcast(io["ecb"], 128, 256, "EB")
    FBc = bias_bcast(io["fcb"], 128, 256, "FB")
    MBc = bias_bcast(io["mlb"], BC, 256, "MB")
    F2Bc = bias_bcast(io["f2b"], BC, 14, "F2B")
    elb_col = bias_col(io["elb"], 128, "elb")
    flb_col = bias_col(io["flb"], 128, "flb")
    mcb_col = bias_col(io["mcb"], 64, "mcb")

    # conv weights, already transposed host-side: 4 tiles [128 i, dh*256+o]
    wtE_all = ctile([128, 4 * 768], BF16, "wtE_all")
    nc.sync.dma_start(wtE_all[:, :].rearrange("p (kc d) -> p kc d", d=768),
                      io["ecwT"].rearrange("(kc p) d -> p kc d", p=128))
    wtE = [wtE_all[:, kc * 768:(kc + 1) * 768] for kc in range(4)]
    wtF_all = ctile([128, 4 * 768], BF16, "wtF_all")
    nc.scalar.dma_start(wtF_all[:, :].rearrange("p (kc d) -> p kc d", d=768),
                        io["fcwT"].rearrange("(kc p) d -> p kc d", p=128))
    wtF = [wtF_all[:, kc * 768:(kc + 1) * 768] for kc in range(4)]

    # manip tap-sum weights, host-transposed: [128 i, {int,h0,hL}*64+o]
    mwT_sb = ctile([128, 192], F32R, "mwT_sb")
    nc.sync.dma_start(mwT_sb[:, :], io["mwT"])
    wsumT = {"int": mwT_sb[:, 0:64], "h0": mwT_sb[:, 64:128], "hL": mwT_sb[:, 128:192]}
    mlws_sb = ctile([64, 768], F32R, "mlws_sb")
    nc.scalar.dma_start(mlws_sb[:, :], io["mlws"])

    # pair-max tables: pm[t0, t1*512+ch] = max(emb[t0,ch], emb[t1,ch]).
    # Built as two partition-base-0 pieces (t0 0..7 / 8..13), then reshaped
    # to [pair, ch] partition chunks by SBUF->SBUF DMA (all on-chip).
    # Transients (flat-broadcast emb + pm halves) live in a scoped pool.
    def pair_table(prep, emb_dram, dt, dma_eng, tag):
        embA = prep.tile([8, EMB], dt, tag="embA", name=tag + "_embA")
        nc.gpsimd.dma_start(embA[:, :], emb_dram[0:8, :])
        embB = prep.tile([6, EMB], dt, tag="embB", name=tag + "_embB")
        nc.gpsimd.dma_start(embB[:, :], emb_dram[8:V, :])
        embF = prep.tile([V, V * EMB], dt, tag="embF", name=tag + "_embF")
        nc.gpsimd.dma_start(
            embF[:, :],
            emb_dram.rearrange("v e -> () (v e)").partition_broadcast(V))
        tps = []
        for half, esb, nt0 in (("0", embA, 8), ("1", embB, 6)):
            pm = prep.tile([nt0, V * EMB], dt, tag="pm", name=tag + "pm" + half)
            for t1 in range(V):
                nc.vector.tensor_tensor(pm[:, t1 * EMB:(t1 + 1) * EMB],
                                        esb[:, :], embF[0:nt0, t1 * EMB:(t1 + 1) * EMB],
                                        ALU.max)
            tp = ctile([nt0 * V, EMB], dt, tag + half)
            nc.gpsimd.dma_start(tp[:, :], pm[:, :])
            tps.append(tp)
        return tps[0], tps[1]

    # ---------------- shared stage helpers ----------------
    def embed_pool_grp(idx_row, g, tp0, tp1, slab_dt, tag):
        """Group g (8 samples): one-hot embed + pair-max -> 4 padded slabs."""
        slabs = [slabp.tile([128, SLABW], slab_dt, tag=f"slab{kc}",
                            name=f"{tag}slab{kc}_{g}") for kc in range(4)]
        npad = SLAB + 1
        for kc in range(4):
            nc.vector.tensor_copy(slabs[kc][:, 0:SLABW:H + 1], zpadb[:, 0:npad])
        npos = SLAB * H  # 1024
        oh0 = ohp.tile([PAIR0, npos], slab_dt, tag="oh0", name=f"{tag}oh0_{g}")
        oh1 = ohp.tile([PAIR1, npos], slab_dt, tag="oh1", name=f"{tag}oh1_{g}")
        for nt in range(npos // 512):
            idxpp = psum_emb.tile([PAIR0, 512], F32, tag="pp", name="idxpp")
            nc.tensor.matmul(idxpp[:, :], ones_row[:, 0:PAIR0],
                             idx_row[:, g * npos + nt * 512:
                                     g * npos + (nt + 1) * 512],
                             start=True, stop=True)
            nc.vector.tensor_scalar(oh0[:, nt * 512:(nt + 1) * 512],
                                    idxpp[:, :], iota_col[0:PAIR0, :],
                                    None, ALU.is_equal)
            nc.vector.tensor_scalar(oh1[:, nt * 512:(nt + 1) * 512],
                                    idxpp[0:PAIR1, :], float(PAIR0),
                                    iota_col[0:PAIR1, :], ALU.subtract,
                                    ALU.is_equal)
        for kc in range(4):
            for nt in range(npos // 512):
                pp = psum_emb.tile([128, 512], F32, tag="pp", name="pp")
                nc.tensor.matmul(pp[:, :], tp0[:, kc * 128:(kc + 1) * 128],
                                 oh0[:, nt * 512:(nt + 1) * 512],
                                 start=True, stop=False)
                nc.tensor.matmul(pp[:, :], tp1[:, kc * 128:(kc + 1) * 128],
                                 oh1[:, nt * 512:(nt + 1) * 512],
                                 start=False, stop=True)
                # scatter 4 samples x 128 positions into the padded slab
                s0 = nt * 4
                dst = slabs[kc][:, 1 + s0 * (H + 1): 1 + (s0 + 4) * (H + 1)] \
                    .rearrange("p (s w) -> p s w", w=H + 1)[:, :, 0:H]
                nc.vector.tensor_copy(
                    dst, pp[:, :].rearrange("p (s w) -> p s w", w=H))
        return slabs

    def conv_grp(slabs, g, wts, bias_bc, acts):
        """3-tap conv for the 8 samples of group g; write biased acts."""
        for ls in range(SLAB):
            s = g * SLAB + ls
            cp = psum_conv.tile([128, 256], F32, tag="cp", name="cp")
            first = True
            for kc in range(4):
                for dh in range(3):
                    lhsT = slabs[kc][:, ls * (H + 1) + dh: ls * (H + 1) + dh + 128]
                    rhs = wts[kc][:, dh * 256:(dh + 1) * 256]
                    nc.tensor.matmul(cp[:, :], lhsT, rhs,
                                     start=first, stop=(kc == 3 and dh == 2))
                    first = False
            nc.vector.tensor_tensor(acts[:, s * 256:(s + 1) * 256],
                                    cp[:, :], bias_bc[:, :], ALU.add)

    def big_linear(acts, w_dram, wdt, pool, dma_eng, grp, rowk, tag):
        """psum[j(128), b(32)] = sum_c W_c^T @ acts[:, (b, o=c)].

        Streams W in batches of `grp` 128-row chunks per DMA (rowk chunks
        packed per DRAM row) through `pool` ring buffers on `dma_eng`.
        """
        lp = psum_lin.tile([128, BC], F32, tag="lp", name=f"{tag}_lp")
        nrows = grp // rowk * 128
        ncols = rowk * 128
        for g in range(256 // grp):
            wsb = pool.tile([128, grp * 128], wdt, tag="w", name=f"{tag}w{g}")
            src = w_dram[g * nrows:(g + 1) * nrows, :] \
                .rearrange("(j p) k -> p j k", p=128)
            dma_eng.dma_start(wsb[:, :].rearrange("p (j k) -> p j k", k=ncols), src)
            for jj in range(grp):
                c = g * grp + jj
                rhs = acts[:, c:c + (BC - 1) * 256 + 1:256]
                nc.tensor.matmul(lp[:, :], wsb[:, jj * 128:(jj + 1) * 128], rhs,
                                 start=(c == 0), stop=(c == 255))
        return lp

    # ---------------- enemy branch ----------------
    xsb = wtile([BC, L], I32, "xsb")
    nc.gpsimd.dma_start(xsb[:, :], io["x"])
    xf = wtile([BC, L], F32, "xf")
    nc.vector.tensor_copy(xf[:, :], xsb[:, :])
    idxE = wtile([BC, H], F32, "idxE")
    nc.vector.scalar_tensor_tensor(idxE[:, :], xf[:, 0:L:2], float(V),
                                   xf[:, 1:L:2], ALU.mult, ALU.add)
    idxrowE = wtile([1, BC * H], F32, "idxrow")
    nc.gpsimd.dma_start(idxrowE[:, :], idxE[:, :])

    actsE = wtile([128, BC * 256], BF16, "acts")
    with tc.tile_pool(name="tblprep", bufs=1) as prep:
        tpE0, tpE1 = pair_table(prep, io["eemb"], BF16, nc.sync, "tpE")
        for g in range(NGRP):
            slabs = embed_pool_grp(idxrowE, g, tpE0, tpE1, BF16, "E")
            conv_grp(slabs, g, wtE, EBc, actsE)
        # friend tables build here: their DMA/vector work hides under the
        # enemy conv phase, and the prep buffers are reused in place.
        tpF0, tpF1 = pair_table(prep, io["femb"], BF16, nc.scalar, "tpF")

    elw_pool = ctx.enter_context(tc.tile_pool(name="elw_pool", bufs=6))
    lpE = big_linear(actsE, io["elw2"], BF16, elw_pool, nc.sync, 16, 2, "E")
    # softmax over j (partition dim): exp, sum via matmul, normalize
    Ex = wtile([128, BC], F32, "Ex")
    nc.scalar.activation(Ex[:, :], lpE[:, :], AF.Exp, bias=elb_col[:, :])
    s1 = psum_sm.tile([BC, 1], F32, tag="sm", name="s1")
    nc.tensor.matmul(s1[:, :], Ex[:, :], ones_col[:, :], start=True, stop=True)
    r32 = wtile([BC, 1], F32, "r32")
    nc.vector.reciprocal(r32[:, :], s1[:, :])
    rrow = wtile([1, BC], F32, "rrow")
    nc.gpsimd.dma_start(rrow[:, :], r32[:, :])
    rbp = psum_sm.tile([128, BC], F32, tag="sm", name="rbp")
    nc.tensor.matmul(rbp[:, :], ones_row[:, :], rrow[:, :], start=True, stop=True)
    vT = wtile([128, BC], F32R, "vT")   # enemy_out^T [i, b]
    nc.vector.tensor_tensor(vT[:, :], Ex[:, :], rbp[:, :], ALU.mult)

    # ---------------- manipulator ----------------
    mp = psum_lin.tile([BC, 256], F32, tag="lp", name="mp")
    for k, name in enumerate(("int", "h0", "hL")):
        cx = psum_sm.tile([64, BC], F32, tag="sm", name="cx")
        nc.tensor.matmul(cx[:, :], wsumT[name],
                         vT[:, :], start=True, stop=True)
        cxs = wtile([64, BC], F32R, f"cxs_{name}")
        nc.scalar.activation(cxs[:, :], cx[:, :], AF.Relu, bias=mcb_col[:, :])
        nc.tensor.matmul(mp[:, :], cxs[:, :],
                         mlws_sb[:, k * 256:(k + 1) * 256],
                         start=(k == 0), stop=(k == 2))
    m_sb = wtile([BC, 256], F32, "m_sb")
    nc.vector.tensor_tensor(m_sb[:, :], mp[:, :], MBc[0:BC, :], ALU.add)

    # tokens = floor(|m|*100) mod 14; pair idx = 14*even + odd
    # floor via the 2^23 magic-number trick (t in [0, ~50) << 2^23):
    #   round_nearest(t - 0.5 + 2^23) - 2^23 == floor(t) for non-integer t
    # mod 14 via repeated conditional subtract (covers t < 42)
    tt = wtile([BC, 256], F32, "tt")
    nc.scalar.activation(tt[:, :], m_sb[:, :], AF.Abs, scale=100.0)
    fu = wtile([BC, 256], F32, "fu")
    nc.vector.tensor_scalar(fu[:, :], tt[:, :], 8388607.5, None, ALU.add)
    fr = wtile([BC, 256], F32, "fr")
    nc.vector.tensor_scalar(fr[:, :], fu[:, :], 8388608.0, None, ALU.subtract)
    ti = wtile([BC, 256], F32, "ti")
    nc.vector.tensor_scalar(ti[:, :], fr[:, :], float(V), None, ALU.is_ge)
    t1 = wtile([BC, 256], F32, "t1")
    nc.vector.scalar_tensor_tensor(t1[:, :], ti[:, :], -float(V), fr[:, :],
                                   ALU.mult, ALU.add)
    t2 = wtile([BC, 256], F32, "t2")
    nc.vector.tensor_scalar(t2[:, :], t1[:, :], float(V), None, ALU.is_ge)
    tok = wtile([BC, 256], F32, "tok")
    nc.vector.scalar_tensor_tensor(tok[:, :], t2[:, :], -float(V), t1[:, :],
                                   ALU.mult, ALU.add)
    idxF = wtile([BC, H], F32, "idxF")
    nc.vector.scalar_tensor_tensor(idxF[:, :], tok[:, 0:256:2], float(V),
                                   tok[:, 1:256:2], ALU.mult, ALU.add)
    idxrowF = wtile([1, BC * H], F32, "idxrow")
    nc.gpsimd.dma_start(idxrowF[:, :], idxF[:, :])

    # ---------------- friend branch (bf16) ----------------
    flw_pool = ctx.enter_context(tc.tile_pool(name="flw_pool", bufs=4))
    actsF = wtile([128, BC * 256], BF16, "acts")
    for g in range(NGRP):
        slabs = embed_pool_grp(idxrowF, g, tpF0, tpF1, BF16, "F")
        conv_grp(slabs, g, wtF, FBc, actsF)

    lpF = big_linear(actsF, io["flw2"], BF16, flw_pool, nc.scalar, 16, 2, "F")
    fsb = wtile([128, BC], F32, "fsb")
    nc.vector.tensor_scalar(fsb[:, :], lpF[:, :], flb_col[:, :], None, ALU.add)

    w2sb = wtile([128, 14], F32, "w2sb")
    nc.gpsimd.dma_start(w2sb[:, :], io["f2w"])
    f2 = psum_sm.tile([BC, 14], F32, tag="sm", name="f2")
    nc.tensor.matmul(f2[:, :], fsb[:, :], w2sb[:, :], start=True, stop=True)
    logits = wtile([BC, 14], F32, "logits")
    nc.vector.tensor_tensor(logits[:, :], f2[:, :], F2Bc[0:BC, :], ALU.add)
    nmx = wtile([BC, 1], F32, "nmx")
    nc.vector.reduce_max(nmx[:, :], logits[:, :], AX.X, negate=True)
    ex = wtile([BC, 14], F32, "ex")
    nc.scalar.activation(ex[:, :], logits[:, :], AF.Exp, bias=nmx[:, :])
    sm = wtile([BC, 1], F32, "sm")
    nc.vector.reduce_sum(sm[:, :], ex[:, :], AX.X)
    rs = wtile([BC, 1], F32, "rs")
    nc.vector.reciprocal(rs[:, :], sm[:, :])
    outt = wtile([BC, 14], F32, "outt")
    nc.vector.tensor_scalar(outt[:, :], ex[:, :], rs[:, :], None, ALU.mult)
    nc.gpsimd.dma_start(io["out"], outt[:, :])


_CACHE = {}


def _get_nc():
    if "nc" not in _CACHE:
        nc = bacc.Bacc("TRN2", target_bir_lowering=False, debug=False,
                       num_devices=NCORES)
        with tile.TileContext(nc) as tc:
            with ExitStack() as ctx:
                build_kernel(nc, tc, ctx)
        nc.compile()
        _CACHE["nc"] = nc
    return _CACHE["nc"]


def prep_inputs(inputs):
    """Host-side shard/layout prep. Returns list of 8 in_maps."""
    f32 = np.float32
    bf16 = ml_dtypes.bfloat16

    ecw = np.asarray(inputs["enemy_conv_w"], f32)[:, :, :, 1]   # [256 o,512 i,3]
    ecwT = np.ascontiguousarray(ecw.transpose(1, 2, 0)).reshape(512, 768) \
        .astype(bf16)
    fcw = np.asarray(inputs["friend_conv_w"], f32)[:, :, :, 1]
    fcwT = np.ascontiguousarray(fcw.transpose(1, 2, 0)).reshape(512, 768) \
        .astype(bf16)
    mcw = np.asarray(inputs["manip_conv_w"], f32)[:, :, :, 1]   # [64 o,128 i,3]
    m_int = (mcw[:, :, 0] + mcw[:, :, 1] + mcw[:, :, 2]).T      # [128 i, 64 o]
    m_h0 = (mcw[:, :, 1] + mcw[:, :, 2]).T
    m_hL = (mcw[:, :, 0] + mcw[:, :, 1]).T
    mwT = np.ascontiguousarray(np.concatenate([m_int, m_h0, m_hL], axis=1))
    mlr = np.asarray(inputs["manip_lin_w"], np.float64).reshape(64, 128, 256)
    mlws = np.ascontiguousarray(np.concatenate(
        [mlr[:, 1:127].sum(1), mlr[:, 0], mlr[:, 127]], axis=1)).astype(f32)
    def pack2(w):  # [32768,128] -> pair-packed [16384,256]
        return np.ascontiguousarray(
            np.asarray(w, f32).reshape(128, 2, 128, 128).transpose(0, 2, 1, 3)
            .reshape(16384, 256)).astype(bf16)

    common = {
        "eemb": np.asarray(inputs["enemy_emb"]).astype(bf16),
        "ecwT": ecwT,
        "ecb": np.ascontiguousarray(inputs["enemy_conv_b"], f32),
        "elw2": pack2(inputs["enemy_lin_w"]),
        "elb": np.ascontiguousarray(inputs["enemy_lin_b"], f32),
        "mwT": mwT,
        "mcb": np.ascontiguousarray(inputs["manip_conv_b"], f32),
        "mlws": mlws,
        "mlb": np.ascontiguousarray(inputs["manip_lin_b"], f32),
        "femb": np.asarray(inputs["friend_emb"]).astype(bf16),
        "fcwT": fcwT,
        "fcb": np.ascontiguousarray(inputs["friend_conv_b"], f32),
        "flw2": pack2(inputs["friend_lin1_w"]),
        "flb": np.ascontiguousarray(inputs["friend_lin1_b"], f32),
        "f2w": np.ascontiguousarray(inputs["friend_lin2_w"], f32),
        "f2b": np.ascontiguousarray(inputs["friend_lin2_b"], f32),
    }
    x = np.ascontiguousarray(inputs["x"], np.int32)
    return [dict(common, x=np.ascontiguousarray(x[c * BC:(c + 1) * BC]))
            for c in range(NCORES)]


def kernel(**inputs):
    nc = _get_nc()
    in_maps = prep_inputs(inputs)
    res = run_bass_kernel_spmd(nc, in_maps, core_ids=list(range(NCORES)))
    return np.concatenate([r["out"] for r in res.results], axis=0)


# revision 33
# speedup vs baseline: 3.0862x; 1.0460x over previous
"""Trainium2 Bass kernel for nn_Network_67388036874689.

Data-parallel over batch: B=256 sharded as 32 samples on each of 8 cores;
all parameters replicated.

Structure exploited (validated against the reference on host):
  - fog_of_war's greedy scan returns arange(B) -> the permutation is identity.
  - conv2d(3x3, pad=1) on [C, H, 1] spatial input only sees kernel column 1
    -> 1D conv over H with 3 taps.
  - Embedding lookup (V=14) followed by pair-maxpool = lookup into a 196-entry
    pairwise-max table, implemented as one-hot matmuls on the PE.
  - The manipulator conv input is constant over H -> collapses to 3 matmuls
    (interior / h=0 / h=127 tap-sum variants).

Performance structure:
  - Conv / tap-sum weight transposes are done host-side in prep_inputs.
  - The three big weight streams (elw 16.8MB, mlw 8.4MB, flw 8.4MB) are
    DMA'd in large batched transfers on the two HWDGE queues (sync carries
    elw, scalar carries mlw then flw) with ring buffers, so they prefetch
    underneath the conv phases instead of gating the linear phases.
  - friend_lin1_w is pair-packed host-side so every DMA descriptor is 512B.

Precision: critical path to the token discretization (enemy branch + manip)
in fp32 / float32r; post-token friend branch in bf16.
"""

import numpy as np
import ml_dtypes
from contextlib import ExitStack

import concourse.bass as bass
import concourse.bacc as bacc
import concourse.mybir as mybir
import concourse.tile as tile
from concourse.bass_utils import run_bass_kernel_spmd

F32 = mybir.dt.float32
F32R = mybir.dt.float32r
BF16 = mybir.dt.bfloat16
I32 = mybir.dt.int32
AF = mybir.ActivationFunctionType
ALU = mybir.AluOpType
AX = mybir.AxisListType

NCORES = 8
B = 256
BC = B // NCORES        # 32 samples per core
L = 256                 # sequence length
V = 14                  # vocab
EMB = 512               # embedding dim
H = L // 2              # 128 pooled positions
NPAIR = V * V           # 196
PAIR0 = 112             # pair-table chunk split: 112 (t0 0..7) + 84 (t0 8..13)
PAIR1 = NPAIR - PAIR0   # 84
SLAB = 8                # samples per embed/pool slab group
NGRP = BC // SLAB       # 4 groups
SLABW = SLAB * (H + 1) + 1   # padded slab width (stride 129 per sample)


def _dram_inputs(nc):
    t = {}

    def inp(name, shape, dt):
        t[name] = nc.dram_tensor(name, list(shape), dt, kind="ExternalInput").ap()

    inp("x", (BC, L), I32)
    inp("tpE", (NPAIR, EMB), BF16)    # host-built pairwise-max table
    inp("ecwT", (512, 3 * 256), BF16)     # [i_global, dh*256+o]
    inp("ecb", (256,), F32)
    inp("elw2", (16384, 256), BF16)       # pair-packed enemy_lin_w
    inp("elb", (128,), F32)
    inp("mwT", (128, 3 * 64), F32R)       # [i, {int,h0,hL}*64+o] tap sums
    inp("mcb", (64,), F32)
    inp("mlws", (64, 3 * 256), F32R)
    inp("mlb", (256,), F32)
    inp("tpF", (NPAIR, EMB), BF16)
    inp("fcwT", (512, 3 * 256), BF16)     # [i_global, dh*256+o]
    inp("fcb", (256,), F32)
    inp("flw2", (16384, 256), BF16)       # pair-packed friend_lin1_w
    inp("flb", (128,), F32)
    inp("f2w", (128, 14), F32)
    inp("f2b", (14,), F32)
    t["out"] = nc.dram_tensor("out", [BC, 14], F32, kind="ExternalOutput").ap()
    return t


def build_kernel(nc, tc, ctx):
    io = _dram_inputs(nc)
    consts = ctx.enter_context(tc.tile_pool(name="consts", bufs=1))
    work = ctx.enter_context(tc.tile_pool(name="work", bufs=1))
    slabp = ctx.enter_context(tc.tile_pool(name="slabp", bufs=2))
    ohp = ctx.enter_context(tc.tile_pool(name="ohp", bufs=1))
    psum_emb = ctx.enter_context(tc.tile_pool(name="psum_emb", bufs=4, space="PSUM"))
    psum_conv = ctx.enter_context(tc.tile_pool(name="psum_conv", bufs=2, space="PSUM"))
    psum_lin = ctx.enter_context(tc.tile_pool(name="psum_lin", bufs=1, space="PSUM"))
    psum_sm = ctx.enter_context(tc.tile_pool(name="psum_sm", bufs=1, space="PSUM"))

    def ctile(shape, dt, tag):
        return consts.tile(shape, dt, tag=tag, name=tag)

    def wtile(shape, dt, tag):
        return work.tile(shape, dt, tag=tag, name=tag)

    # ---------------- constants ----------------
    iota_i = ctile([128, 1], I32, "iota_i")
    nc.gpsimd.iota(iota_i[:, :], pattern=[[0, 1]], base=0, channel_multiplier=1)
    iota_col = ctile([128, 1], F32, "iota_col")
    nc.vector.tensor_copy(iota_col[:, :], iota_i[:, :])
    ones_col = ctile([128, 1], F32, "ones_col")
    nc.vector.memset(ones_col[:, :], 1.0)
    ones_row = ctile([1, 128], F32, "ones_row")
    nc.vector.memset(ones_row[:, :], 1.0)
    iota_row = ctile([1, 128], F32, "iota_row")
    nc.gpsimd.dma_start(iota_row[:, :], iota_col[:, :])
    e0_row = ctile([1, 128], F32, "e0_row")
    nc.vector.tensor_scalar(e0_row[:, :], iota_row[:, :], 0.0, None, ALU.is_equal)
    eL_row = ctile([1, 128], F32, "eL_row")
    nc.vector.tensor_scalar(eL_row[:, :], iota_row[:, :], 127.0, None, ALU.is_equal)
    ei_row = ctile([1, 128], F32, "ei_row")
    nc.vector.scalar_tensor_tensor(ei_row[:, :], e0_row[:, :], -1.0, eL_row[:, :],
                                   ALU.mult, ALU.subtract)
    nc.vector.tensor_scalar(ei_row[:, :], ei_row[:, :], 1.0, None, ALU.add)
    zpadb = ctile([128, 32], BF16, "zpadb")
    nc.vector.memset(zpadb[:, :], 0.0)

    def bias_col(dram_vec, n, tag):
        col = ctile([n, 1], F32, tag)
        nc.scalar.dma_start(col[:, :], dram_vec)
        return col

    def bias_bcast(dram_vec, rows, width, tag):
        out = ctile([rows, width], F32, tag)
        nc.gpsimd.dma_start(out[:, :], dram_vec[None, :].partition_broadcast(rows))
        return out

    EBc = bias_bcast(io["ecb"], 128, 256, "EB")
    FBc = bias_bcast(io["fcb"], 128, 256, "FB")
    MBc = bias_bcast(io["mlb"], BC, 256, "MB")
    F2Bc = bias_bcast(io["f2b"], BC, 14, "F2B")
    elb_col = bias_col(io["elb"], 128, "elb")
    flb_col = bias_col(io["flb"], 128, "flb")
    mcb_col = bias_col(io["mcb"], 64, "mcb")

    # conv weights, already transposed host-side: 4 tiles [128 i, dh*256+o]
    wtE_all = ctile([128, 4 * 768], BF16, "wtE_all")
    nc.sync.dma_start(wtE_all[:, :].rearrange("p (kc d) -> p kc d", d=768),
                      io["ecwT"].rearrange("(kc p) d -> p kc d", p=128))
    wtE = [wtE_all[:, kc * 768:(kc + 1) * 768] for kc in range(4)]
    wtF_all = ctile([128, 4 * 768], BF16, "wtF_all")
    nc.scalar.dma_start(wtF_all[:, :].rearrange("p (kc d) -> p kc d", d=768),
                        io["fcwT"].rearrange("(kc p) d -> p kc d", p=128))
    wtF = [wtF_all[:, kc * 768:(kc + 1) * 768] for kc in range(4)]

    # manip tap-sum weights, host-transposed: [128 i, {int,h0,hL}*64+o]
    mwT_sb = ctile([128, 192], F32R, "mwT_sb")
    nc.sync.dma_start(mwT_sb[:, :], io["mwT"])
    wsumT = {"int": mwT_sb[:, 0:64], "h0": mwT_sb[:, 64:128], "hL": mwT_sb[:, 128:192]}
    mlws_sb = ctile([64, 768], F32R, "mlws_sb")
    nc.scalar.dma_start(mlws_sb[:, :], io["mlws"])

    # pair-max tables, host-precomputed: tp[t0*V+t1, ch] = max(emb[t0],emb[t1])
    def pair_table(dram, dma_eng, tag):
        tp0 = ctile([PAIR0, EMB], BF16, tag + "0")
        dma_eng.dma_start(tp0[:, :], dram[0:PAIR0, :])
        tp1 = ctile([PAIR1, EMB], BF16, tag + "1")
        dma_eng.dma_start(tp1[:, :], dram[PAIR0:NPAIR, :])
        return tp0, tp1

    # ---------------- shared stage helpers ----------------
    def embed_pool_grp(idx_row, g, tp0, tp1, slab_dt, tag):
        """Group g (8 samples): one-hot embed + pair-max -> 4 padded slabs."""
        slabs = [slabp.tile([128, SLABW], slab_dt, tag=f"slab{kc}",
                            name=f"{tag}slab{kc}_{g}") for kc in range(4)]
        npad = SLAB + 1
        for kc in range(4):
            nc.vector.tensor_copy(slabs[kc][:, 0:SLABW:H + 1], zpadb[:, 0:npad])
        npos = SLAB * H  # 1024
        oh0 = ohp.tile([PAIR0, npos], slab_dt, tag="oh0", name=f"{tag}oh0_{g}")
        oh1 = ohp.tile([PAIR1, npos], slab_dt, tag="oh1", name=f"{tag}oh1_{g}")
        for nt in range(npos // 512):
            idxpp = psum_emb.tile([PAIR0, 512], F32, tag="pp", name="idxpp")
            nc.tensor.matmul(idxpp[:, :], ones_row[:, 0:PAIR0],
                             idx_row[:, g * npos + nt * 512:
                                     g * npos + (nt + 1) * 512],
                             start=True, stop=True)
            nc.vector.tensor_scalar(oh0[:, nt * 512:(nt + 1) * 512],
                                    idxpp[:, :], iota_col[0:PAIR0, :],
                                    None, ALU.is_equal)
            nc.vector.tensor_scalar(oh1[:, nt * 512:(nt + 1) * 512],
                                    idxpp[0:PAIR1, :], float(PAIR0),
                                    iota_col[0:PAIR1, :], ALU.subtract,
                                    ALU.is_equal)
        for kc in range(4):
            for nt in range(npos // 512):
                pp = psum_emb.tile([128, 512], F32, tag="pp", name="pp")
                nc.tensor.matmul(pp[:, :], tp0[:, kc * 128:(kc + 1) * 128],
                                 oh0[:, nt * 512:(nt + 1) * 512],
                                 start=True, stop=False)
                nc.tensor.matmul(pp[:, :], tp1[:, kc * 128:(kc + 1) * 128],
                                 oh1[:, nt * 512:(nt + 1) * 512],
                                 start=False, stop=True)
                # scatter 4 samples x 128 positions into the padded slab
                s0 = nt * 4
                dst = slabs[kc][:, 1 + s0 * (H + 1): 1 + (s0 + 4) * (H + 1)] \
                    .rearrange("p (s w) -> p s w", w=H + 1)[:, :, 0:H]
                nc.vector.tensor_copy(
                    dst, pp[:, :].rearrange("p (s w) -> p s w", w=H))
        return slabs

    def conv_grp(slabs, g, wts, bias_bc, acts):
        """3-tap conv for the 8 samples of group g; write biased acts."""
        for ls in range(SLAB):
            s = g * SLAB + ls
            cp = psum_conv.tile([128, 256], F32, tag="cp", name="cp")
            first = True
            for kc in range(4):
                for dh in range(3):
                    lhsT = slabs[kc][:, ls * (H + 1) + dh: ls * (H + 1) + dh + 128]
                    rhs = wts[kc][:, dh * 256:(dh + 1) * 256]
                    nc.tensor.matmul(cp[:, :], lhsT, rhs,
                                     start=first, stop=(kc == 3 and dh == 2))
                    first = False
            nc.vector.tensor_tensor(acts[:, s * 256:(s + 1) * 256],
                                    cp[:, :], bias_bc[:, :], ALU.add)

    def big_linear(acts, w_dram, wdt, pool, dma_eng, grp, rowk, tag):
        """psum[j(128), b(32)] = sum_c W_c^T @ acts[:, (b, o=c)].

        Streams W in batches of `grp` 128-row chunks per DMA (rowk chunks
        packed per DRAM row) through `pool` ring buffers on `dma_eng`.
        """
        lp = psum_lin.tile([128, BC], F32, tag="lp", name=f"{tag}_lp")
        nrows = grp // rowk * 128
        ncols = rowk * 128
        for g in range(256 // grp):
            wsb = pool.tile([128, grp * 128], wdt, tag="w", name=f"{tag}w{g}")
            src = w_dram[g * nrows:(g + 1) * nrows, :] \
                .rearrange("(j p) k -> p j k", p=128)
            dma_eng.dma_start(wsb[:, :].rearrange("p (j k) -> p j k", k=ncols), src)
            for jj in range(grp):
                c = g * grp + jj
                rhs = acts[:, c:c + (BC - 1) * 256 + 1:256]
                nc.tensor.matmul(lp[:, :], wsb[:, jj * 128:(jj + 1) * 128], rhs,
                                 start=(c == 0), stop=(c == 255))
        return lp

    # ---------------- enemy branch ----------------
    xsb = wtile([BC, L], I32, "xsb")
    nc.gpsimd.dma_start(xsb[:, :], io["x"])
    xf = wtile([BC, L], F32, "xf")
    nc.vector.tensor_copy(xf[:, :], xsb[:, :])
    idxE = wtile([BC, H], F32, "idxE")
    nc.vector.scalar_tensor_tensor(idxE[:, :], xf[:, 0:L:2], float(V),
                                   xf[:, 1:L:2], ALU.mult, ALU.add)
    idxrowE = wtile([1, BC * H], F32, "idxrow")
    nc.gpsimd.dma_start(idxrowE[:, :], idxE[:, :])

    actsE = wtile([128, BC * 256], BF16, "acts")
    tpE0, tpE1 = pair_table(io["tpE"], nc.sync, "tpE")
    tpF0, tpF1 = pair_table(io["tpF"], nc.scalar, "tpF")
    for g in range(NGRP):
        slabs = embed_pool_grp(idxrowE, g, tpE0, tpE1, BF16, "E")
        conv_grp(slabs, g, wtE, EBc, actsE)

    elw_pool = ctx.enter_context(tc.tile_pool(name="elw_pool", bufs=6))
    lpE = big_linear(actsE, io["elw2"], BF16, elw_pool, nc.sync, 16, 2, "E")
    # softmax over j (partition dim): exp, sum via matmul, normalize
    Ex = wtile([128, BC], F32, "Ex")
    nc.scalar.activation(Ex[:, :], lpE[:, :], AF.Exp, bias=elb_col[:, :])
    s1 = psum_sm.tile([BC, 1], F32, tag="sm", name="s1")
    nc.tensor.matmul(s1[:, :], Ex[:, :], ones_col[:, :], start=True, stop=True)
    r32 = wtile([BC, 1], F32, "r32")
    nc.vector.reciprocal(r32[:, :], s1[:, :])
    rrow = wtile([1, BC], F32, "rrow")
    nc.gpsimd.dma_start(rrow[:, :], r32[:, :])
    rbp = psum_sm.tile([128, BC], F32, tag="sm", name="rbp")
    nc.tensor.matmul(rbp[:, :], ones_row[:, :], rrow[:, :], start=True, stop=True)
    vT = wtile([128, BC], F32R, "vT")   # enemy_out^T [i, b]
    nc.vector.tensor_tensor(vT[:, :], Ex[:, :], rbp[:, :], ALU.mult)

    # ---------------- manipulator ----------------
    mp = psum_lin.tile([BC, 256], F32, tag="lp", name="mp")
    for k, name in enumerate(("int", "h0", "hL")):
        cx = psum_sm.tile([64, BC], F32, tag="sm", name="cx")
        nc.tensor.matmul(cx[:, :], wsumT[name],
                         vT[:, :], start=True, stop=True)
        cxs = wtile([64, BC], F32R, f"cxs_{name}")
        nc.scalar.activation(cxs[:, :], cx[:, :], AF.Relu, bias=mcb_col[:, :])
        nc.tensor.matmul(mp[:, :], cxs[:, :],
                         mlws_sb[:, k * 256:(k + 1) * 256],
                         start=(k == 0), stop=(k == 2))
    m_sb = wtile([BC, 256], F32, "m_sb")
    nc.vector.tensor_tensor(m_sb[:, :], mp[:, :], MBc[0:BC, :], ALU.add)

    # tokens = floor(|m|*100) mod 14; pair idx = 14*even + odd
    # floor via the 2^23 magic-number trick (t in [0, ~50) << 2^23):
    #   round_nearest(t - 0.5 + 2^23) - 2^23 == floor(t) for non-integer t
    # mod 14 via repeated conditional subtract (covers t < 42)
    tt = wtile([BC, 256], F32, "tt")
    nc.scalar.activation(tt[:, :], m_sb[:, :], AF.Abs, scale=100.0)
    fu = wtile([BC, 256], F32, "fu")
    nc.vector.tensor_scalar(fu[:, :], tt[:, :], 8388607.5, None, ALU.add)
    fr = wtile([BC, 256], F32, "fr")
    nc.vector.tensor_scalar(fr[:, :], fu[:, :], 8388608.0, None, ALU.subtract)
    ti = wtile([BC, 256], F32, "ti")
    nc.vector.tensor_scalar(ti[:, :], fr[:, :], float(V), None, ALU.is_ge)
    t1 = wtile([BC, 256], F32, "t1")
    nc.vector.scalar_tensor_tensor(t1[:, :], ti[:, :], -float(V), fr[:, :],
                                   ALU.mult, ALU.add)
    t2 = wtile([BC, 256], F32, "t2")
    nc.vector.tensor_scalar(t2[:, :], t1[:, :], float(V), None, ALU.is_ge)
    tok = wtile([BC, 256], F32, "tok")
    nc.vector.scalar_tensor_tensor(tok[:, :], t2[:, :], -float(V), t1[:, :],
                                   ALU.mult, ALU.add)
    idxF = wtile([BC, H], F32, "idxF")
    nc.vector.scalar_tensor_tensor(idxF[:, :], tok[:, 0:256:2], float(V),
                                   tok[:, 1:256:2], ALU.mult, ALU.add)
    idxrowF = wtile([1, BC * H], F32, "idxrow")
    nc.gpsimd.dma_start(idxrowF[:, :], idxF[:, :])

    # ---------------- friend branch (bf16) ----------------
    flw_pool = ctx.enter_context(tc.tile_pool(name="flw_pool", bufs=4))
    actsF = wtile([128, BC * 256], BF16, "acts")
    for g in range(NGRP):
        slabs = embed_pool_grp(idxrowF, g, tpF0, tpF1, BF16, "F")
        conv_grp(slabs, g, wtF, FBc, actsF)

    lpF = big_linear(actsF, io["flw2"], BF16, flw_pool, nc.scalar, 16, 2, "F")
    fsb = wtile([128, BC], F32, "fsb")
    nc.vector.tensor_scalar(fsb[:, :], lpF[:, :], flb_col[:, :], None, ALU.add)

    w2sb = wtile([128, 14], F32, "w2sb")
    nc.gpsimd.dma_start(w2sb[:, :], io["f2w"])
    f2 = psum_sm.tile([BC, 14], F32, tag="sm", name="f2")
    nc.tensor.matmul(f2[:, :], fsb[:, :], w2sb[:, :], start=True, stop=True)
    logits = wtile([BC, 14], F32, "logits")
    nc.vector.tensor_tensor(logits[:, :], f2[:, :], F2Bc[0:BC, :], ALU.add)
    nmx = wtile([BC, 1], F32, "nmx")
    nc.vector.reduce_max(nmx[:, :], logits[:, :], AX.X, negate=True)
    ex = wtile([BC, 14], F32, "ex")
    nc.scalar.activation(ex[:, :], logits[:, :], AF.Exp, bias=nmx[:, :])
    sm = wtile([BC, 1], F32, "sm")
    nc.vector.reduce_sum(sm[:, :], ex[:, :], AX.X)
    rs = wtile([BC, 1], F32, "rs")
    nc.vector.reciprocal(rs[:, :], sm[:, :])
    outt = wtile([BC, 14], F32, "outt")
    nc.vector.tensor_scalar(outt[:, :], ex[:, :], rs[:, :], None, ALU.mult)
    nc.gpsimd.dma_start(io["out"], outt[:, :])


_CACHE = {}


def _get_nc():
    if "nc" not in _CACHE:
        nc = bacc.Bacc("TRN2", target_bir_lowering=False, debug=False,
                       num_devices=NCORES)
        with tile.TileContext(nc) as tc:
            with ExitStack() as ctx:
                build_kernel(nc, tc, ctx)
        nc.compile()
        _CACHE["nc"] = nc
    return _CACHE["nc"]


def prep_inputs(inputs):
    """Host-side shard/layout prep. Returns list of 8 in_maps."""
    f32 = np.float32
    bf16 = ml_dtypes.bfloat16

    ecw = np.asarray(inputs["enemy_conv_w"], f32)[:, :, :, 1]   # [256 o,512 i,3]
    ecwT = np.ascontiguousarray(ecw.transpose(1, 2, 0)).reshape(512, 768) \
        .astype(bf16)
    fcw = np.asarray(inputs["friend_conv_w"], f32)[:, :, :, 1]
    fcwT = np.ascontiguousarray(fcw.transpose(1, 2, 0)).reshape(512, 768) \
        .astype(bf16)
    mcw = np.asarray(inputs["manip_conv_w"], f32)[:, :, :, 1]   # [64 o,128 i,3]
    m_int = (mcw[:, :, 0] + mcw[:, :, 1] + mcw[:, :, 2]).T      # [128 i, 64 o]
    m_h0 = (mcw[:, :, 1] + mcw[:, :, 2]).T
    m_hL = (mcw[:, :, 0] + mcw[:, :, 1]).T
    mwT = np.ascontiguousarray(np.concatenate([m_int, m_h0, m_hL], axis=1))
    mlr = np.asarray(inputs["manip_lin_w"], np.float64).reshape(64, 128, 256)
    mlws = np.ascontiguousarray(np.concatenate(
        [mlr[:, 1:127].sum(1), mlr[:, 0], mlr[:, 127]], axis=1)).astype(f32)
    def pairtab(emb):  # [14,512] -> [196,512] pairwise max, bf16
        e = np.asarray(emb, f32).astype(bf16)
        return np.ascontiguousarray(
            np.maximum(e[:, None, :], e[None, :, :]).reshape(NPAIR, EMB))

    def pack2(w):  # [32768,128] -> pair-packed [16384,256]
        return np.ascontiguousarray(
            np.asarray(w, f32).reshape(128, 2, 128, 128).transpose(0, 2, 1, 3)
            .reshape(16384, 256)).astype(bf16)

    common = {
        "tpE": pairtab(inputs["enemy_emb"]),
        "ecwT": ecwT,
        "ecb": np.ascontiguousarray(inputs["enemy_conv_b"], f32),
        "elw2": pack2(inputs["enemy_lin_w"]),
        "elb": np.ascontiguousarray(inputs["enemy_lin_b"], f32),
        "mwT": mwT,
        "mcb": np.ascontiguousarray(inputs["manip_conv_b"], f32),
        "mlws": mlws,
        "mlb": np.ascontiguousarray(inputs["manip_lin_b"], f32),
        "tpF": pairtab(inputs["friend_emb"]),
        "fcwT": fcwT,
        "fcb": np.ascontiguousarray(inputs["friend_conv_b"], f32),
        "flw2": pack2(inputs["friend_lin1_w"]),
        "flb": np.ascontiguousarray(inputs["friend_lin1_b"], f32),
        "f2w": np.ascontiguousarray(inputs["friend_lin2_w"], f32),
        "f2b": np.ascontiguousarray(inputs["friend_lin2_b"], f32),
    }
    x = np.ascontiguousarray(inputs["x"], np.int32)
    return [dict(common, x=np.ascontiguousarray(x[c * BC:(c + 1) * BC]))
            for c in range(NCORES)]


def kernel(**inputs):
    nc = _get_nc()
    in_maps = prep_inputs(inputs)
    res = run_bass_kernel_spmd(nc, in_maps, core_ids=list(range(NCORES)))
    return np.concatenate([r["out"] for r in res.results], axis=0)
